# revision 11
# baseline (speedup 1.0000x reference)
"""MoE transformer block on 8 Trainium2 cores.

Layer: x = x + attn(ln1(x)); x = x + moe(ln2(x)).
Shapes: B=4, T=1024, C=768, H=12 heads, E=8 experts, top-2, cap=1280, F=3072.

Distribution:
  Launch A (attention): core i -> batch i//2, heads 6*(i%2) .. +6.
    LN1 affine is folded into the QKV weights host-side; each core emits a
    partial (6-head) output projection, transposed [C, T], f32. Host sums the
    two half-head partials per batch and adds the residual.
  Host: ln2 + gating + exact top-2 capacity routing (numpy, matches the jax
    reference bit-for-bit in ordering), builds per-expert gather indices.
  Launch B (experts): core e -> expert e. xbT [C, cap] bf16 in, outT [C, cap]
    f32 out. Host scatter-adds w * out into y (per-expert indices are unique,
    so fancy-index += is collision-free).
"""

import math

import numpy as np
import ml_dtypes

import concourse.bacc as bacc
import concourse.bass as bass
import concourse.mybir as mybir
import concourse.tile as tile
from concourse import bass_utils
from concourse.masks import make_identity

F32 = mybir.dt.float32
BF16 = mybir.dt.bfloat16
AF = mybir.ActivationFunctionType
ALU = mybir.AluOpType
AX = mybir.AxisListType

B, T, C = 4, 1024, 768
NHEAD = 12
HD = C // NHEAD  # 64
E = 8
TOPK = 2
CAP = 1280
F = 4 * C  # 3072
LN_EPS = 1e-5
NEG_INF = -1e30
P = 128

N_CORES = 8
H6 = NHEAD // 2          # heads per core
D6 = H6 * HD             # 384
CSUB = C // P            # 6
KSUB_F = F // P          # 24

_CACHE = {}


# --------------------------------------------------------------------------
# Launch A: attention
# --------------------------------------------------------------------------

def build_attn():
    nc = bacc.Bacc("TRN2", target_bir_lowering=False, debug=False)

    xb = nc.dram_tensor("xb", [T, C], F32, kind="ExternalInput")
    # qkv weight slice for this core's 6 heads, ln1-folded, q pre-scaled by
    # 1/sqrt(HD). column order: q h0..h5 | k h0..h5 | v h0..h5  (64 cols each)
    wqkv = nc.dram_tensor("wqkv", [C, 3 * D6], BF16, kind="ExternalInput")
    bqkv = nc.dram_tensor("bqkv", [P, 3 * D6 // P], F32, kind="ExternalInput")
    wpj = nc.dram_tensor("wpj", [D6, C], BF16, kind="ExternalInput")
    bpj = nc.dram_tensor("bpj", [P, CSUB], F32, kind="ExternalInput")
    # transposed causal mask: cmaskT[k, q] = 0 if k <= q else -1e30
    cmaskT = nc.dram_tensor("cmaskT", [P, P], F32, kind="ExternalInput")
    out = nc.dram_tensor("attn_pT", [C, T], F32, kind="ExternalOutput")

    NT = T // P  # 8 token tiles
    QKV9 = 3 * D6 // P  # 9

    with tile.TileContext(nc) as tc:
        with (
            tc.tile_pool(name="const", bufs=1) as const,
            tc.tile_pool(name="xin", bufs=3) as xin,
            tc.tile_pool(name="big", bufs=1) as big,
            tc.tile_pool(name="pTp", bufs=2) as pTp,
            tc.tile_pool(name="work", bufs=3) as work,
            tc.tile_pool(name="ps", bufs=4, space="PSUM") as ps,
            tc.tile_pool(name="ps_t", bufs=2, space="PSUM") as ps_t,
            tc.tile_pool(name="ps_y", bufs=2, space="PSUM") as ps_y,
        ):
            ident = const.tile([P, P], BF16)
            make_identity(nc, ident[:])
            cm = const.tile([P, P], F32)
            nc.sync.dma_start(cm[:], cmaskT[:])
            wqkv_sb = const.tile([P, CSUB, 3 * D6], BF16)
            nc.sync.dma_start(
                wqkv_sb[:], wqkv[:].rearrange("(o p) n -> p o n", p=P))
            bqkv_sb = const.tile([P, QKV9], F32)
            nc.sync.dma_start(bqkv_sb[:], bqkv[:])
            wpj_sb = const.tile([P, D6 // P, C], BF16)
            nc.sync.dma_start(
                wpj_sb[:], wpj[:].rearrange("(o p) n -> p o n", p=P))
            bpj_sb = const.tile([P, CSUB], F32)
            nc.sync.dma_start(bpj_sb[:], bpj[:])

            # ---- LN1 (affine folded into wqkv) + transpose -> xlnT [C, T]
            xlnT = big.tile([P, CSUB, T], BF16)
            for ti in range(NT):
                xt = xin.tile([P, C], F32)
                nc.sync.dma_start(xt[:], xb[ti * P:(ti + 1) * P, :])
                s1 = work.tile([P, 1], F32, tag="s1")
                nc.vector.reduce_sum(s1[:], xt[:], axis=AX.X)
                sq = work.tile([P, C], F32, tag="sq")
                s2 = work.tile([P, 1], F32, tag="s2")
                nc.scalar.activation(sq[:], xt[:], AF.Square, accum_out=s2[:])
                negmu = work.tile([P, 1], F32, tag="negmu")
                nc.vector.tensor_scalar_mul(negmu[:], s1[:], -1.0 / C)
                # var = s2/C - mu^2
                var = work.tile([P, 1], F32, tag="var")
                nc.vector.tensor_scalar_mul(var[:], s2[:], 1.0 / C)
                mu2 = work.tile([P, 1], F32, tag="mu2")
                nc.vector.tensor_tensor(mu2[:], negmu[:], negmu[:], op=ALU.mult)
                nc.vector.tensor_tensor(var[:], var[:], mu2[:], op=ALU.subtract)
                nc.vector.tensor_scalar_add(var[:], var[:], LN_EPS)
                std = work.tile([P, 1], F32, tag="std")
                nc.scalar.activation(std[:], var[:], AF.Sqrt)
                rstd = work.tile([P, 1], F32, tag="rstd")
                nc.vector.reciprocal(rstd[:], std[:])
                xn = work.tile([P, C], BF16, tag="xn")
                nc.vector.tensor_scalar(
                    xn[:], xt[:], negmu[:], rstd[:], op0=ALU.add, op1=ALU.mult)
                for cs in range(CSUB):
                    pt = ps_t.tile([P, P], BF16)
                    nc.tensor.transpose(
                        pt[:], xn[:, cs * P:(cs + 1) * P], ident[:])
                    nc.vector.tensor_copy(
                        xlnT[:, cs, ti * P:(ti + 1) * P], pt[:])

            # ---- qkvT [3*D6, T] = wqkv.T @ xln.T, + bias
            qkvT = big.tile([P, QKV9, T], BF16)
            for mc in range(QKV9):
                for th in range(T // 512):
                    pacc = ps.tile([P, 512], F32, tag="mm")
                    for ks in range(CSUB):
                        nc.tensor.matmul(
                            pacc[:],
                            lhsT=wqkv_sb[:, ks, mc * P:(mc + 1) * P],
                            rhs=xlnT[:, ks, th * 512:(th + 1) * 512],
                            start=(ks == 0), stop=(ks == CSUB - 1))
                    nc.scalar.activation(
                        qkvT[:, mc, th * 512:(th + 1) * 512], pacc[:],
                        AF.Identity, bias=bqkv_sb[:, mc:mc + 1])

            # ---- v_ones [k_in, k_block, head, 1+64] from vT rows: col 0 is
            # all-ones (fused softmax-denominator column), cols 1..65 = v_h.
            v_ones = big.tile([P, NT, H6, 1 + 64], BF16)
            nc.vector.memset(v_ones[:, :, :, 0:1], 1.0)
            for ti in range(NT):
                for j in range(D6 // P):
                    pt = ps_t.tile([P, P], BF16)
                    nc.tensor.transpose(
                        pt[:], qkvT[:, 2 * (D6 // P) + j, ti * P:(ti + 1) * P],
                        ident[:])
                    nc.vector.tensor_copy(
                        v_ones[:, ti, 2 * j, 1:], pt[:, :64])
                    nc.vector.tensor_copy(
                        v_ones[:, ti, 2 * j + 1, 1:], pt[:, 64:])

            # ---- attention: scores computed transposed, sT[k, q], so Exp
            # lands pT in SBUF directly; AV fuses the denominator via col 0.
            y_big = big.tile([P, NT, D6], BF16)
            for h in range(H6):
                qp0 = 64 * (h % 2)
                qrow = h // 2
                kp0 = (D6 + 64 * h) % P
                krow = (D6 + 64 * h) // P
                pT = pTp.tile([P, NT, T], BF16, tag="pT")
                for kb in range(NT):
                    q0 = kb * P
                    spans = []
                    if q0 < 512:
                        spans.append((q0, 512 - q0))
                    if True:
                        s0 = max(q0, 512)
                        spans.append((s0, T - s0))
                    for (s0, w) in spans:
                        if w <= 0:
                            continue
                        pscore = ps.tile([P, 512], F32, tag="mm")
                        nc.tensor.matmul(
                            pscore[:, :w],
                            lhsT=qkvT[kp0:kp0 + 64, krow, kb * P:(kb + 1) * P],
                            rhs=qkvT[qp0:qp0 + 64, qrow, s0:s0 + w],
                            start=True, stop=True)
                        if s0 <= q0 < s0 + w:
                            off = q0 - s0
                            nc.vector.tensor_tensor(
                                pscore[:, off:off + P], pscore[:, off:off + P],
                                cm[:], op=ALU.add)
                        nc.scalar.activation(
                            pT[:, kb, s0:s0 + w], pscore[:, :w], AF.Exp)
                for qi in range(NT):
                    py = ps_y.tile([P, 65], F32)
                    for kb in range(qi + 1):
                        nc.tensor.matmul(
                            py[:], lhsT=pT[:, kb, qi * P:(qi + 1) * P],
                            rhs=v_ones[:, kb, h, :],
                            start=(kb == 0), stop=(kb == qi))
                    rec = work.tile([P, 1], F32, tag="rec")
                    nc.vector.reciprocal(rec[:], py[:, 0:1])
                    nc.vector.tensor_tensor(
                        y_big[:, qi, h * 64:(h + 1) * 64], py[:, 1:],
                        rec[:].to_broadcast([P, 64]), op=ALU.mult)

            # ---- yT [D6, T]
            yT = big.tile([P, D6 // P, T], BF16)
            for qi in range(NT):
                for j in range(D6 // P):
                    pt = ps_t.tile([P, P], BF16)
                    nc.tensor.transpose(
                        pt[:], y_big[:, qi, j * P:(j + 1) * P], ident[:])
                    nc.vector.tensor_copy(yT[:, j, qi * P:(qi + 1) * P], pt[:])

            # ---- partial projection: outT [C, T] = wpj.T @ y.T + bpj
            for cc in range(CSUB):
                o_sb = work.tile([P, T], F32, tag="osb")
                for th in range(T // 512):
                    pacc = ps.tile([P, 512], F32, tag="mm")
                    for j in range(D6 // P):
                        nc.tensor.matmul(
                            pacc[:],
                            lhsT=wpj_sb[:, j, cc * P:(cc + 1) * P],
                            rhs=yT[:, j, th * 512:(th + 1) * 512],
                            start=(j == 0), stop=(j == D6 // P - 1))
                    nc.scalar.activation(
                        o_sb[:, th * 512:(th + 1) * 512], pacc[:],
                        AF.Identity, bias=bpj_sb[:, cc:cc + 1])
                nc.sync.dma_start(out[cc * P:(cc + 1) * P, :], o_sb[:])

    nc.compile()
    return nc


# --------------------------------------------------------------------------
# Launch B: experts
# --------------------------------------------------------------------------

def build_expert():
    nc = bacc.Bacc("TRN2", target_bir_lowering=False, debug=False)

    xbT = nc.dram_tensor("xbT", [C, CAP], BF16, kind="ExternalInput")
    fcw = nc.dram_tensor("fcw", [C, F], BF16, kind="ExternalInput")
    fcb = nc.dram_tensor("fcb", [P, KSUB_F], F32, kind="ExternalInput")
    pjw = nc.dram_tensor("pjw", [F, C], BF16, kind="ExternalInput")
    pjb = nc.dram_tensor("pjb", [P, CSUB], F32, kind="ExternalInput")
    out = nc.dram_tensor("outT", [C, CAP], F32, kind="ExternalOutput")

    SC = [(0, 512), (512, 512), (1024, 256)]  # slot chunks of CAP=1280

    with tile.TileContext(nc) as tc:
        with (
            tc.tile_pool(name="const", bufs=1) as const,
            tc.tile_pool(name="w1", bufs=6) as w1p,
            tc.tile_pool(name="w2", bufs=4) as w2p,
            tc.tile_pool(name="big", bufs=1) as big,
            tc.tile_pool(name="osb", bufs=2) as osbp,
            tc.tile_pool(name="ps", bufs=4, space="PSUM") as ps,
        ):
            xbT_sb = const.tile([P, CSUB, CAP], BF16)
            nc.sync.dma_start(
                xbT_sb[:], xbT[:].rearrange("(o p) n -> p o n", p=P))
            fcb_sb = const.tile([P, KSUB_F], F32)
            nc.sync.dma_start(fcb_sb[:], fcb[:])
            pjb_sb = const.tile([P, CSUB], F32)
            nc.sync.dma_start(pjb_sb[:], pjb[:])

            fcw_r = fcw[:].rearrange("(o p) n -> p o n", p=P)
            pjw_r = pjw[:].rearrange("(o p) n -> p o n", p=P)

            hT = big.tile([P, KSUB_F, CAP], BF16)
            for mf in range(KSUB_F):
                wt = w1p.tile([P, CSUB, P], BF16, tag="w1")
                nc.sync.dma_start(wt[:], fcw_r[:, :, mf * P:(mf + 1) * P])
                for (s0, sw) in SC:
                    pacc = ps.tile([P, 512], F32, tag="mm")
                    for ks in range(CSUB):
                        nc.tensor.matmul(
                            pacc[:, :sw], lhsT=wt[:, ks, :],
                            rhs=xbT_sb[:, ks, s0:s0 + sw],
                            start=(ks == 0), stop=(ks == CSUB - 1))
                    nc.scalar.activation(
                        hT[:, mf, s0:s0 + sw], pacc[:, :sw],
                        AF.Gelu, bias=fcb_sb[:, mf:mf + 1])

            for cc in range(CSUB):
                wt = w2p.tile([P, KSUB_F, P], BF16, tag="w2")
                nc.sync.dma_start(wt[:], pjw_r[:, :, cc * P:(cc + 1) * P])
                o_sb = osbp.tile([P, CAP], F32, tag="osb")
                for (s0, sw) in SC:
                    pacc = ps.tile([P, 512], F32)
                    for ks in range(KSUB_F):
                        nc.tensor.matmul(
                            pacc[:, :sw], lhsT=wt[:, ks, :],
                            rhs=hT[:, ks, s0:s0 + sw],
                            start=(ks == 0), stop=(ks == KSUB_F - 1))
                    nc.scalar.activation(
                        o_sb[:, s0:s0 + sw], pacc[:, :sw],
                        AF.Identity, bias=pjb_sb[:, cc:cc + 1])
                nc.sync.dma_start(out[cc * P:(cc + 1) * P, :], o_sb[:])

    nc.compile()
    return nc


# --------------------------------------------------------------------------
# Host glue
# --------------------------------------------------------------------------

def _bf16(a):
    return np.asarray(a, np.float32).astype(ml_dtypes.bfloat16)


def _pcol(vec, nsub):
    """[nsub*P] -> [P, nsub] per-partition bias layout."""
    return np.ascontiguousarray(
        np.asarray(vec, np.float32).reshape(nsub, P).T)


def _layer_norm(x, w, b):
    mu = x.mean(-1, keepdims=True)
    var = x.var(-1, keepdims=True)
    return (x - mu) / np.sqrt(var + LN_EPS) * w + b


def _exact_logits(need, x, ln1_w, ln1_b, ln2_w, ln2_b, qkv_w, qkv_b,
                  proj_w, proj_b, w_g):
    """fp32 gating logits for the given flat token indices (exact attention
    rows for just those tokens)."""
    out = np.empty((need.size, E), np.float32)
    bs, ps = need // T, need % T
    for b in np.unique(bs):
        m = bs == b
        pos = ps[m]                              # [M]
        xl = _layer_norm(x[b], ln1_w, ln1_b)     # [T, C]
        kv = xl @ qkv_w[:, C:] + qkv_b[C:]       # [T, 2C]
        k = kv[:, :C].reshape(T, NHEAD, HD)
        v = kv[:, C:].reshape(T, NHEAD, HD)
        q = (xl[pos] @ qkv_w[:, :C] + qkv_b[:C]).reshape(-1, NHEAD, HD)
        s = np.einsum("mhd,khd->mhk", q, k) / math.sqrt(HD)
        s = np.where(pos[:, None, None] >= np.arange(T)[None, None, :],
                     s, NEG_INF)
        s -= s.max(-1, keepdims=True)
        p = np.exp(s)
        p /= p.sum(-1, keepdims=True)
        y = np.einsum("mhk,khd->mhd", p, v).reshape(-1, C)
        att = y @ proj_w + proj_b
        x2 = x[b][pos] + att
        out[m] = _layer_norm(x2, ln2_w, ln2_b) @ w_g
    return out


def kernel(x, ln1_w, ln1_b, ln2_w, ln2_b, attn_qkv_w, attn_qkv_b,
           attn_proj_w, attn_proj_b, w_g, exp_fc_w, exp_fc_b,
           exp_proj_w, exp_proj_b):
    x = np.asarray(x, np.float32)
    ln1_w = np.asarray(ln1_w, np.float32)
    ln1_b = np.asarray(ln1_b, np.float32)
    attn_qkv_w = np.asarray(attn_qkv_w, np.float32)
    attn_qkv_b = np.asarray(attn_qkv_b, np.float32)
    attn_proj_w = np.asarray(attn_proj_w, np.float32)
    attn_proj_b = np.asarray(attn_proj_b, np.float32)

    if "attn" not in _CACHE:
        _CACHE["attn"] = build_attn()
    if "expert" not in _CACHE:
        _CACHE["expert"] = build_expert()

    # ---------------- launch A ----------------
    # fold ln1 affine into qkv: qkv = xhat @ (diag(w1) W) + (b1 @ W + b)
    Wf = ln1_w[:, None] * attn_qkv_w          # [C, 3C]
    bf = ln1_b @ attn_qkv_w + attn_qkv_b      # [3C]
    # fold 1/sqrt(HD) into q columns
    Wq = Wf[:, :C] / math.sqrt(HD)
    bq = bf[:C] / math.sqrt(HD)
    Wk, bk = Wf[:, C:2 * C], bf[C:2 * C]
    Wv, bv = Wf[:, 2 * C:], bf[2 * C:]

    cmaskT_np = np.where(
        np.triu(np.ones((P, P), bool)), 0.0, NEG_INF).astype(np.float32)

    in_maps_a = []
    for core in range(N_CORES):
        b = core // 2
        h0 = H6 * (core % 2)
        cols = slice(h0 * HD, (h0 + H6) * HD)
        wqkv_c = np.concatenate([Wq[:, cols], Wk[:, cols], Wv[:, cols]], 1)
        bqkv_c = np.concatenate([bq[cols], bk[cols], bv[cols]])
        bpj_c = attn_proj_b if core % 2 == 0 else np.zeros(C, np.float32)
        in_maps_a.append({
            "xb": np.ascontiguousarray(x[b]),
            "wqkv": _bf16(wqkv_c),
            "bqkv": _pcol(bqkv_c, 3 * D6 // P),
            "wpj": _bf16(attn_proj_w[h0 * HD:(h0 + H6) * HD, :]),
            "bpj": _pcol(bpj_c, CSUB),
            "cmaskT": cmaskT_np,
        })

    res_a = bass_utils.run_bass_kernel_spmd(
        _CACHE["attn"], in_maps_a, core_ids=list(range(N_CORES)))

    attn = np.empty((B, T, C), np.float32)
    for b in range(B):
        attn[b] = (res_a.results[2 * b]["attn_pT"]
                   + res_a.results[2 * b + 1]["attn_pT"]).T

    x2 = x + attn                       # [B, T, C]
    xf2 = x2.reshape(B * T, C)

    # ---------------- host routing (exact reference semantics) -------------
    N = B * T
    xln2 = _layer_norm(xf2, np.asarray(ln2_w, np.float32),
                       np.asarray(ln2_b, np.float32))
    logits = xln2 @ np.asarray(w_g, np.float32)        # [N, E]

    # The top-2 expert choice is discontinuous: tokens whose top2/top3 gating
    # logits are within the bf16 noise floor could route differently than the
    # fp32 reference would. Recompute those few tokens' logits exactly.
    srt = np.sort(logits, axis=1)
    need = np.nonzero(srt[:, -2] - srt[:, -3] < 0.02)[0]
    if need.size:
        logits[need] = _exact_logits(
            need, x, ln1_w, ln1_b, np.asarray(ln2_w, np.float32),
            np.asarray(ln2_b, np.float32), attn_qkv_w, attn_qkv_b,
            attn_proj_w, attn_proj_b, np.asarray(w_g, np.float32))

    order = np.argsort(-logits, axis=1, kind="stable")
    topk_idx = order[:, :TOPK]                          # [N, K]
    sel = np.zeros((N, E), bool)
    np.put_along_axis(sel, topk_idx, True, axis=1)
    masked = np.where(sel, logits, NEG_INF)
    m = masked.max(1, keepdims=True)
    ex = np.exp(masked - m)
    router_probs = ex / ex.sum(1, keepdims=True)        # [N, E]

    # capacity ranks in (k, n) order
    exp_mask = np.zeros((TOPK, N, E), np.int64)
    kk = np.arange(TOPK)[:, None]
    nn = np.arange(N)[None, :]
    exp_mask[kk, nn, topk_idx.T] = 1
    flat = exp_mask.reshape(TOPK * N, E)
    rank = np.cumsum(flat, axis=0) - 1                  # [K*N, E]
    keep = (flat == 1) & (rank < CAP)
    kpos, epos = np.nonzero(keep)
    token = kpos % N
    slot = rank[kpos, epos]
    wgt = router_probs[token, epos]

    idx_e = np.zeros((E, CAP), np.int64)
    w_e = np.zeros((E, CAP), np.float32)
    idx_e[epos, slot] = token
    w_e[epos, slot] = wgt

    # ---------------- launch B ----------------
    xln2_bf = _bf16(xln2)
    exp_fc_w = np.asarray(exp_fc_w, np.float32)
    exp_fc_b = np.asarray(exp_fc_b, np.float32).reshape(E, F)
    exp_proj_w = np.asarray(exp_proj_w, np.float32)
    exp_proj_b = np.asarray(exp_proj_b, np.float32).reshape(E, C)

    in_maps_b = []
    for e in range(E):
        xbT = np.ascontiguousarray(xln2_bf[idx_e[e]].T)     # [C, CAP]
        in_maps_b.append({
            "xbT": xbT,
            "fcw": _bf16(exp_fc_w[e]),
            "fcb": _pcol(exp_fc_b[e], KSUB_F),
            "pjw": _bf16(exp_proj_w[e]),
            "pjb": _pcol(exp_proj_b[e], CSUB),
        })

    res_b = bass_utils.run_bass_kernel_spmd(
        _CACHE["expert"], in_maps_b, core_ids=list(range(N_CORES)))

    y = xf2.copy()
    for e in range(E):
        valid = w_e[e] != 0
        y[idx_e[e, valid]] += (w_e[e, valid, None]
                               * res_b.results[e]["outT"].T[valid])
    return y.reshape(B, T, C).astype(np.float32)


# revision 15
# speedup vs baseline: 1.1435x; 1.1435x over previous
"""MoE transformer block on 8 Trainium2 cores.

Layer: x = x + attn(ln1(x)); x = x + moe(ln2(x)).
Shapes: B=4, T=1024, C=768, H=12 heads, E=8 experts, top-2, cap=1280, F=3072.

Distribution:
  Launch A (attention): core i -> batch i//2, heads 6*(i%2) .. +6.
    LN1 affine is folded into the QKV weights host-side; each core emits a
    partial (6-head) output projection, transposed [C, T], f32. Host sums the
    two half-head partials per batch and adds the residual.
  Host: ln2 + gating + exact top-2 capacity routing (numpy, matches the jax
    reference in ordering; near-tie tokens get exact fp32 logits), builds
    per-expert gather indices.
  Launch B (experts): core e -> expert e, slots packed to the observed max
    load (rounded up to 64). xbT [C, cap_k] bf16 in, outT [C, cap_k] f32 out.
    Host scatter-adds w * out into y (per-expert indices are unique, so
    fancy-index += is collision-free).
"""

import math

import numpy as np
import ml_dtypes

import concourse.bacc as bacc
import concourse.bass as bass
import concourse.mybir as mybir
import concourse.tile as tile
from concourse import bass_utils
from concourse.masks import make_identity

F32 = mybir.dt.float32
BF16 = mybir.dt.bfloat16
AF = mybir.ActivationFunctionType
ALU = mybir.AluOpType
AX = mybir.AxisListType

B, T, C = 4, 1024, 768
NHEAD = 12
HD = C // NHEAD  # 64
E = 8
TOPK = 2
CAP = 1280
F = 4 * C  # 3072
LN_EPS = 1e-5
NEG_INF = -1e30
P = 128

N_CORES = 8
H6 = NHEAD // 2          # heads per core
D6 = H6 * HD             # 384
CSUB = C // P            # 6
KSUB_F = F // P          # 24
NT = T // P              # 8
QKV9 = 3 * D6 // P       # 9

_CACHE = {}


def _chunks(n, step=512):
    out = []
    s = 0
    while s < n:
        out.append((s, min(step, n - s)))
        s += step
    return out


# --------------------------------------------------------------------------
# Launch A: attention
# --------------------------------------------------------------------------

def build_attn():
    nc = bacc.Bacc("TRN2", target_bir_lowering=False, debug=False)

    xb = nc.dram_tensor("xb", [T, C], F32, kind="ExternalInput")
    # qkv weight slice for this core's 6 heads, ln1-folded, q pre-scaled by
    # 1/sqrt(HD), pre-permuted to [p, ks, n]. column order within n:
    # q h0..h5 | k h0..h5 | v h0..h5 (64 cols each head)
    wqkv = nc.dram_tensor("wqkv", [P, CSUB, 3 * D6], BF16, kind="ExternalInput")
    bqkv = nc.dram_tensor("bqkv", [P, QKV9], F32, kind="ExternalInput")
    wpj = nc.dram_tensor("wpj", [P, D6 // P, C], BF16, kind="ExternalInput")
    bpj = nc.dram_tensor("bpj", [P, CSUB], F32, kind="ExternalInput")
    # transposed causal mask (bf16): cmaskT[k, q] = 0 if k <= q else -1e30
    cmaskT = nc.dram_tensor("cmaskT", [P, P], BF16, kind="ExternalInput")
    out = nc.dram_tensor("attn_pT", [C, T], F32, kind="ExternalOutput")

    with tile.TileContext(nc) as tc:
        with (
            tc.tile_pool(name="const", bufs=1) as const,
            tc.tile_pool(name="xin", bufs=1) as xin,
            tc.tile_pool(name="big", bufs=1) as big,
            tc.tile_pool(name="pTp", bufs=2) as pTp,
            tc.tile_pool(name="work", bufs=3) as work,
            tc.tile_pool(name="ps", bufs=2, space="PSUM") as ps,
            tc.tile_pool(name="ps_t", bufs=2, space="PSUM") as ps_t,
            tc.tile_pool(name="ps_y", bufs=2, space="PSUM") as ps_y,
        ):
            ident = const.tile([P, P], BF16)
            make_identity(nc, ident[:])
            cm = const.tile([P, P], BF16)
            nc.sync.dma_start(cm[:], cmaskT[:])
            wqkv_sb = const.tile([P, CSUB, 3 * D6], BF16)
            nc.sync.dma_start(wqkv_sb[:], wqkv[:])
            bqkv_sb = const.tile([P, QKV9], F32)
            nc.sync.dma_start(bqkv_sb[:], bqkv[:])
            wpj_sb = const.tile([P, D6 // P, C], BF16)
            nc.sync.dma_start(wpj_sb[:], wpj[:])
            bpj_sb = const.tile([P, CSUB], F32)
            nc.sync.dma_start(bpj_sb[:], bpj[:])

            # ---- LN1 stats for all tiles first (keeps ACT on one table set)
            xts, rstds, negmus = [], [], []
            for ti in range(NT):
                xt = xin.tile([P, C], F32, tag=f"x{ti}")
                nc.sync.dma_start(xt[:], xb[ti * P:(ti + 1) * P, :])
                s1 = work.tile([P, 1], F32, tag="s1")
                nc.vector.reduce_sum(s1[:], xt[:], axis=AX.X)
                sq = work.tile([P, C], F32, tag="sq")
                s2 = work.tile([P, 1], F32, tag="s2")
                nc.scalar.activation(sq[:], xt[:], AF.Square, accum_out=s2[:])
                negmu = xin.tile([P, 1], F32, tag=f"nm{ti}")
                nc.vector.tensor_scalar_mul(negmu[:], s1[:], -1.0 / C)
                var = work.tile([P, 1], F32, tag="var")
                nc.vector.tensor_scalar_mul(var[:], s2[:], 1.0 / C)
                mu2 = work.tile([P, 1], F32, tag="mu2")
                nc.vector.tensor_tensor(mu2[:], negmu[:], negmu[:], op=ALU.mult)
                nc.vector.tensor_tensor(var[:], var[:], mu2[:], op=ALU.subtract)
                nc.vector.tensor_scalar_add(var[:], var[:], LN_EPS)
                std = work.tile([P, 1], F32, tag="std")
                nc.scalar.activation(std[:], var[:], AF.Sqrt)
                rstd = xin.tile([P, 1], F32, tag=f"rs{ti}")
                nc.vector.reciprocal(rstd[:], std[:])
                xts.append(xt)
                rstds.append(rstd)
                negmus.append(negmu)

            # ---- normalize + transpose -> xlnT [C, T]
            xlnT = big.tile([P, CSUB, T], BF16)
            for ti in range(NT):
                xn = work.tile([P, C], BF16, tag="xn")
                nc.vector.tensor_scalar(
                    xn[:], xts[ti][:], negmus[ti][:], rstds[ti][:],
                    op0=ALU.add, op1=ALU.mult)
                pt = ps_t.tile([P, CSUB, P], BF16, tag="pt6")
                for cs in range(CSUB):
                    nc.tensor.transpose(
                        pt[:, cs, :], xn[:, cs * P:(cs + 1) * P], ident[:])
                nc.vector.tensor_copy(xlnT[:, :, ti * P:(ti + 1) * P], pt[:])

            # ---- qkvT [3*D6, T] = wqkv.T @ xln.T, + bias
            qkvT = big.tile([P, QKV9, T], BF16)
            for mc in range(QKV9):
                pacc = ps.tile([P, T], F32, tag="mm")
                for th in range(T // 512):
                    for ks in range(CSUB):
                        nc.tensor.matmul(
                            pacc[:, th * 512:(th + 1) * 512],
                            lhsT=wqkv_sb[:, ks, mc * P:(mc + 1) * P],
                            rhs=xlnT[:, ks, th * 512:(th + 1) * 512],
                            start=(ks == 0), stop=(ks == CSUB - 1))
                nc.scalar.activation(
                    qkvT[:, mc, :], pacc[:],
                    AF.Identity, bias=bqkv_sb[:, mc:mc + 1])

            # ---- v_ones [k_in, k_block, head, 1+64] from vT rows: col 0 is
            # all-ones (fused softmax-denominator column), cols 1..65 = v_h.
            v_ones = big.tile([P, NT, H6, 1 + 64], BF16)
            nc.vector.memset(v_ones[:, :, :, 0:1], 1.0)
            for ti in range(NT):
                pt6 = ps_t.tile([P, CSUB, P], BF16, tag="pt6")
                pt = pt6[:, :D6 // P, :]
                for j in range(D6 // P):
                    nc.tensor.transpose(
                        pt[:, j, :],
                        qkvT[:, 2 * (D6 // P) + j, ti * P:(ti + 1) * P],
                        ident[:])
                nc.vector.tensor_copy(
                    v_ones[:, ti, :, 1:],
                    pt[:].rearrange("p j (a b) -> p (j a) b", a=2))

            # ---- attention: scores computed transposed, sT[k, q], so Exp
            # lands pT in SBUF directly; AV fuses the denominator via col 0.
            # The causal mask of the diagonal block is accumulated into PSUM
            # by the PE itself (I.T @ cmaskT).
            y_big = big.tile([P, NT, D6], BF16)
            for h in range(H6):
                qp0 = 64 * (h % 2)
                qrow = h // 2
                kp0 = (D6 + 64 * h) % P
                krow = (D6 + 64 * h) // P
                pT = pTp.tile([P, NT, T], BF16, tag="pT")
                for kb in range(NT):
                    q0 = kb * P
                    span = T - q0
                    pscore = ps.tile([P, T], F32, tag="mm")
                    # chunk on absolute 512 boundaries (PSUM bank alignment)
                    bounds = [q0] + [b for b in (512, T) if b > q0]
                    for (s0, e0) in zip(bounds[:-1], bounds[1:]):
                        w = e0 - s0
                        nc.tensor.matmul(
                            pscore[:, s0:s0 + w],
                            lhsT=qkvT[kp0:kp0 + 64, krow, kb * P:(kb + 1) * P],
                            rhs=qkvT[qp0:qp0 + 64, qrow, s0:s0 + w],
                            start=True, stop=True)
                    nc.tensor.matmul(
                        pscore[:, q0:q0 + P], lhsT=ident[:], rhs=cm[:],
                        start=False, stop=True, skip_group_check=True)
                    nc.scalar.activation(
                        pT[:, kb, q0:], pscore[:, q0:], AF.Exp)
                for qi in range(NT):
                    py = ps_y.tile([P, 65], F32)
                    for kb in range(qi + 1):
                        nc.tensor.matmul(
                            py[:], lhsT=pT[:, kb, qi * P:(qi + 1) * P],
                            rhs=v_ones[:, kb, h, :],
                            start=(kb == 0), stop=(kb == qi))
                    rec = work.tile([P, 1], F32, tag="rec")
                    nc.vector.reciprocal(rec[:], py[:, 0:1])
                    nc.vector.tensor_tensor(
                        y_big[:, qi, h * 64:(h + 1) * 64], py[:, 1:],
                        rec[:].to_broadcast([P, 64]), op=ALU.mult)

            # ---- yT [D6, T]
            yT = big.tile([P, D6 // P, T], BF16)
            for qi in range(NT):
                pt6 = ps_t.tile([P, CSUB, P], BF16, tag="pt6")
                pt = pt6[:, :D6 // P, :]
                for j in range(D6 // P):
                    nc.tensor.transpose(
                        pt[:, j, :], y_big[:, qi, j * P:(j + 1) * P], ident[:])
                nc.vector.tensor_copy(yT[:, :, qi * P:(qi + 1) * P], pt[:])

            # ---- partial projection: outT [C, T] = wpj.T @ y.T + bpj
            for cc in range(CSUB):
                pacc = ps.tile([P, T], F32, tag="mm")
                o_sb = work.tile([P, T], F32, tag="osb")
                for th in range(T // 512):
                    for j in range(D6 // P):
                        nc.tensor.matmul(
                            pacc[:, th * 512:(th + 1) * 512],
                            lhsT=wpj_sb[:, j, cc * P:(cc + 1) * P],
                            rhs=yT[:, j, th * 512:(th + 1) * 512],
                            start=(j == 0), stop=(j == D6 // P - 1))
                nc.scalar.activation(
                    o_sb[:], pacc[:], AF.Identity, bias=bpj_sb[:, cc:cc + 1])
                nc.sync.dma_start(out[cc * P:(cc + 1) * P, :], o_sb[:])

    nc.compile()
    return nc


# --------------------------------------------------------------------------
# Launch B: experts
# --------------------------------------------------------------------------

def build_expert(cap_k):
    nc = bacc.Bacc("TRN2", target_bir_lowering=False, debug=False)

    xbT = nc.dram_tensor("xbT", [P, CSUB, cap_k], BF16, kind="ExternalInput")
    fcw = nc.dram_tensor("fcw", [KSUB_F, P, CSUB, P], BF16,
                         kind="ExternalInput")
    fcb = nc.dram_tensor("fcb", [P, KSUB_F], F32, kind="ExternalInput")
    pjw = nc.dram_tensor("pjw", [CSUB, P, KSUB_F, P], BF16,
                         kind="ExternalInput")
    pjb = nc.dram_tensor("pjb", [P, CSUB], F32, kind="ExternalInput")
    out = nc.dram_tensor("outT", [C, cap_k], F32, kind="ExternalOutput")

    SC = _chunks(cap_k)

    with tile.TileContext(nc) as tc:
        with (
            tc.tile_pool(name="const", bufs=1) as const,
            tc.tile_pool(name="w1", bufs=6) as w1p,
            tc.tile_pool(name="w2", bufs=4) as w2p,
            tc.tile_pool(name="big", bufs=1) as big,
            tc.tile_pool(name="osb", bufs=2) as osbp,
            tc.tile_pool(name="ps", bufs=4, space="PSUM") as ps,
        ):
            xbT_sb = const.tile([P, CSUB, cap_k], BF16)
            nc.sync.dma_start(xbT_sb[:], xbT[:])
            fcb_sb = const.tile([P, KSUB_F], F32)
            nc.sync.dma_start(fcb_sb[:], fcb[:])
            pjb_sb = const.tile([P, CSUB], F32)
            nc.sync.dma_start(pjb_sb[:], pjb[:])

            hT = big.tile([P, KSUB_F, cap_k], BF16)
            for mf in range(KSUB_F):
                wt = w1p.tile([P, CSUB, P], BF16, tag="w1")
                nc.sync.dma_start(wt[:], fcw[mf])
                for (s0, sw) in SC:
                    pacc = ps.tile([P, 512], F32, tag="mm")
                    for ks in range(CSUB):
                        nc.tensor.matmul(
                            pacc[:, :sw], lhsT=wt[:, ks, :],
                            rhs=xbT_sb[:, ks, s0:s0 + sw],
                            start=(ks == 0), stop=(ks == CSUB - 1))
                    nc.scalar.activation(
                        hT[:, mf, s0:s0 + sw], pacc[:, :sw],
                        AF.Gelu, bias=fcb_sb[:, mf:mf + 1])

            for cc in range(CSUB):
                wt = w2p.tile([P, KSUB_F, P], BF16, tag="w2")
                nc.sync.dma_start(wt[:], pjw[cc])
                o_sb = osbp.tile([P, cap_k], F32, tag="osb")
                for (s0, sw) in SC:
                    pacc = ps.tile([P, 512], F32, tag="mm")
                    for ks in range(KSUB_F):
                        nc.tensor.matmul(
                            pacc[:, :sw], lhsT=wt[:, ks, :],
                            rhs=hT[:, ks, s0:s0 + sw],
                            start=(ks == 0), stop=(ks == KSUB_F - 1))
                    nc.scalar.activation(
                        o_sb[:, s0:s0 + sw], pacc[:, :sw],
                        AF.Identity, bias=pjb_sb[:, cc:cc + 1])
                nc.sync.dma_start(out[cc * P:(cc + 1) * P, :], o_sb[:])

    nc.compile()
    return nc


# --------------------------------------------------------------------------
# Host glue
# --------------------------------------------------------------------------

def _bf16(a):
    return np.asarray(a, np.float32).astype(ml_dtypes.bfloat16)


def _pcol(vec, nsub):
    """[nsub*P] -> [P, nsub] per-partition bias layout."""
    return np.ascontiguousarray(
        np.asarray(vec, np.float32).reshape(nsub, P).T)


def _kperm(w):
    """[K, N] -> [P, K//P, N] partition-major layout, contiguous."""
    k, n = w.shape
    return np.ascontiguousarray(w.reshape(k // P, P, n).transpose(1, 0, 2))


def _layer_norm(x, w, b):
    mu = x.mean(-1, keepdims=True)
    var = x.var(-1, keepdims=True)
    return (x - mu) / np.sqrt(var + LN_EPS) * w + b


def _exact_logits(need, x, ln1_w, ln1_b, ln2_w, ln2_b, qkv_w, qkv_b,
                  proj_w, proj_b, w_g):
    """fp32 gating logits for the given flat token indices (exact attention
    rows for just those tokens)."""
    out = np.empty((need.size, E), np.float32)
    bs, ps = need // T, need % T
    for b in np.unique(bs):
        m = bs == b
        pos = ps[m]                              # [M]
        xl = _layer_norm(x[b], ln1_w, ln1_b)     # [T, C]
        kv = xl @ qkv_w[:, C:] + qkv_b[C:]       # [T, 2C]
        k = kv[:, :C].reshape(T, NHEAD, HD)
        v = kv[:, C:].reshape(T, NHEAD, HD)
        q = (xl[pos] @ qkv_w[:, :C] + qkv_b[:C]).reshape(-1, NHEAD, HD)
        s = np.einsum("mhd,khd->mhk", q, k) / math.sqrt(HD)
        s = np.where(pos[:, None, None] >= np.arange(T)[None, None, :],
                     s, NEG_INF)
        s -= s.max(-1, keepdims=True)
        p = np.exp(s)
        p /= p.sum(-1, keepdims=True)
        y = np.einsum("mhk,khd->mhd", p, v).reshape(-1, C)
        att = y @ proj_w + proj_b
        x2 = x[b][pos] + att
        out[m] = _layer_norm(x2, ln2_w, ln2_b) @ w_g
    return out


def kernel(x, ln1_w, ln1_b, ln2_w, ln2_b, attn_qkv_w, attn_qkv_b,
           attn_proj_w, attn_proj_b, w_g, exp_fc_w, exp_fc_b,
           exp_proj_w, exp_proj_b):
    x = np.asarray(x, np.float32)
    ln1_w = np.asarray(ln1_w, np.float32)
    ln1_b = np.asarray(ln1_b, np.float32)
    attn_qkv_w = np.asarray(attn_qkv_w, np.float32)
    attn_qkv_b = np.asarray(attn_qkv_b, np.float32)
    attn_proj_w = np.asarray(attn_proj_w, np.float32)
    attn_proj_b = np.asarray(attn_proj_b, np.float32)

    if "attn" not in _CACHE:
        _CACHE["attn"] = build_attn()

    # ---------------- launch A ----------------
    # fold ln1 affine into qkv: qkv = xhat @ (diag(w1) W) + (b1 @ W + b)
    Wf = ln1_w[:, None] * attn_qkv_w          # [C, 3C]
    bf = ln1_b @ attn_qkv_w + attn_qkv_b      # [3C]
    Wq = Wf[:, :C] / math.sqrt(HD)
    bq = bf[:C] / math.sqrt(HD)
    Wk, bk = Wf[:, C:2 * C], bf[C:2 * C]
    Wv, bv = Wf[:, 2 * C:], bf[2 * C:]

    cmaskT_np = _bf16(np.where(
        np.triu(np.ones((P, P), bool)), 0.0, NEG_INF))

    in_maps_a = []
    for core in range(N_CORES):
        b = core // 2
        h0 = H6 * (core % 2)
        cols = slice(h0 * HD, (h0 + H6) * HD)
        wqkv_c = np.concatenate([Wq[:, cols], Wk[:, cols], Wv[:, cols]], 1)
        bqkv_c = np.concatenate([bq[cols], bk[cols], bv[cols]])
        bpj_c = attn_proj_b if core % 2 == 0 else np.zeros(C, np.float32)
        in_maps_a.append({
            "xb": np.ascontiguousarray(x[b]),
            "wqkv": _kperm(_bf16(wqkv_c)),
            "bqkv": _pcol(bqkv_c, QKV9),
            "wpj": _kperm(_bf16(attn_proj_w[h0 * HD:(h0 + H6) * HD, :])),
            "bpj": _pcol(bpj_c, CSUB),
            "cmaskT": cmaskT_np,
        })

    res_a = bass_utils.run_bass_kernel_spmd(
        _CACHE["attn"], in_maps_a, core_ids=list(range(N_CORES)))

    attn = np.empty((B, T, C), np.float32)
    for b in range(B):
        attn[b] = (res_a.results[2 * b]["attn_pT"]
                   + res_a.results[2 * b + 1]["attn_pT"]).T

    x2 = x + attn                       # [B, T, C]
    xf2 = x2.reshape(B * T, C)

    # ---------------- host routing (exact reference semantics) -------------
    N = B * T
    xln2 = _layer_norm(xf2, np.asarray(ln2_w, np.float32),
                       np.asarray(ln2_b, np.float32))
    logits = xln2 @ np.asarray(w_g, np.float32)        # [N, E]

    # The top-2 expert choice is discontinuous: tokens whose top2/top3 gating
    # logits are within the bf16 noise floor could route differently than the
    # fp32 reference would. Recompute those few tokens' logits exactly.
    srt = np.sort(logits, axis=1)
    need = np.nonzero(srt[:, -2] - srt[:, -3] < 0.02)[0]
    if need.size:
        logits[need] = _exact_logits(
            need, x, ln1_w, ln1_b, np.asarray(ln2_w, np.float32),
            np.asarray(ln2_b, np.float32), attn_qkv_w, attn_qkv_b,
            attn_proj_w, attn_proj_b, np.asarray(w_g, np.float32))

    order = np.argsort(-logits, axis=1, kind="stable")
    topk_idx = order[:, :TOPK]                          # [N, K]
    sel = np.zeros((N, E), bool)
    np.put_along_axis(sel, topk_idx, True, axis=1)
    masked = np.where(sel, logits, NEG_INF)
    m = masked.max(1, keepdims=True)
    ex = np.exp(masked - m)
    router_probs = ex / ex.sum(1, keepdims=True)        # [N, E]

    # capacity ranks in (k, n) order
    exp_mask = np.zeros((TOPK, N, E), np.int64)
    kk = np.arange(TOPK)[:, None]
    nn = np.arange(N)[None, :]
    exp_mask[kk, nn, topk_idx.T] = 1
    flat = exp_mask.reshape(TOPK * N, E)
    rank = np.cumsum(flat, axis=0) - 1                  # [K*N, E]
    keep = (flat == 1) & (rank < CAP)
    kpos, epos = np.nonzero(keep)
    token = kpos % N
    slot = rank[kpos, epos]
    wgt = router_probs[token, epos]

    max_load = int(np.bincount(epos, minlength=E).max())
    cap_k = min(CAP, max(64, -(-max_load // 64) * 64))
    if ("expert", cap_k) not in _CACHE:
        _CACHE[("expert", cap_k)] = build_expert(cap_k)

    idx_e = np.zeros((E, cap_k), np.int64)
    w_e = np.zeros((E, cap_k), np.float32)
    idx_e[epos, slot] = token
    w_e[epos, slot] = wgt

    # ---------------- launch B ----------------
    xln2_bf = _bf16(xln2)
    exp_fc_w = np.asarray(exp_fc_w, np.float32)
    exp_fc_b = np.asarray(exp_fc_b, np.float32).reshape(E, F)
    exp_proj_w = np.asarray(exp_proj_w, np.float32)
    exp_proj_b = np.asarray(exp_proj_b, np.float32).reshape(E, C)

    in_maps_b = []
    for e in range(E):
        xbT = _kperm(np.ascontiguousarray(xln2_bf[idx_e[e]].T))
        fcw = _bf16(exp_fc_w[e]).reshape(CSUB, P, KSUB_F, P)
        fcw = np.ascontiguousarray(fcw.transpose(2, 1, 0, 3))
        pjw = _bf16(exp_proj_w[e]).reshape(KSUB_F, P, CSUB, P)
        pjw = np.ascontiguousarray(pjw.transpose(2, 1, 0, 3))
        in_maps_b.append({
            "xbT": xbT,
            "fcw": fcw,
            "fcb": _pcol(exp_fc_b[e], KSUB_F),
            "pjw": pjw,
            "pjb": _pcol(exp_proj_b[e], CSUB),
        })

    res_b = bass_utils.run_bass_kernel_spmd(
        _CACHE[("expert", cap_k)], in_maps_b, core_ids=list(range(N_CORES)))

    y = xf2.copy()
    for e in range(E):
        valid = w_e[e] != 0
        y[idx_e[e, valid]] += (w_e[e, valid, None]
                               * res_b.results[e]["outT"].T[valid])
    return y.reshape(B, T, C).astype(np.float32)


# revision 24
# speedup vs baseline: 1.1878x; 1.0388x over previous
"""MoE transformer block on 8 Trainium2 cores.

Layer: x = x + attn(ln1(x)); x = x + moe(ln2(x)).
Shapes: B=4, T=1024, C=768, H=12 heads, E=8 experts, top-2, cap=1280, F=3072.

Distribution:
  Launch A (attention): core i -> batch i//2, heads 6*(i%2) .. +6.
    LN1 affine is folded into the QKV weights host-side; each core emits a
    partial (6-head) output projection, transposed [C, T], f32. Host sums the
    two half-head partials per batch and adds the residual.
  Host: ln2 + gating + exact top-2 capacity routing (numpy, matches the jax
    reference in ordering; near-tie tokens get exact fp32 logits), builds
    per-expert gather indices.
  Launch B (experts): core e -> expert e, slots packed to the observed max
    load (rounded up to 64). xbT [C, cap_k] bf16 in, outT [C, cap_k] f32 out.
    Host scatter-adds w * out into y (per-expert indices are unique, so
    fancy-index += is collision-free).
"""

import math

import numpy as np
import ml_dtypes

import concourse.bacc as bacc
import concourse.bass as bass
import concourse.mybir as mybir
import concourse.tile as tile
from concourse import bass_utils
from concourse.masks import make_identity

F32 = mybir.dt.float32
BF16 = mybir.dt.bfloat16
AF = mybir.ActivationFunctionType
ALU = mybir.AluOpType
AX = mybir.AxisListType

B, T, C = 4, 1024, 768
NHEAD = 12
HD = C // NHEAD  # 64
E = 8
TOPK = 2
CAP = 1280
F = 4 * C  # 3072
LN_EPS = 1e-5
NEG_INF = -1e30
P = 128

N_CORES = 8
H6 = NHEAD // 2          # heads per core
D6 = H6 * HD             # 384
CSUB = C // P            # 6
KSUB_F = F // P          # 24
NT = T // P              # 8
QKV9 = 3 * D6 // P       # 9

_CACHE = {}


def _chunks(n, step=512):
    out = []
    s = 0
    while s < n:
        out.append((s, min(step, n - s)))
        s += step
    return out


# --------------------------------------------------------------------------
# Launch A: attention
# --------------------------------------------------------------------------

def build_attn():
    nc = bacc.Bacc("TRN2", target_bir_lowering=False, debug=False)

    xb = nc.dram_tensor("xb", [T, C], F32, kind="ExternalInput")
    # qkv weight slice for this core's 6 heads, ln1-folded, q pre-scaled by
    # 1/sqrt(HD), pre-permuted to [p, ks, n]. column order within n:
    # q h0..h5 | k h0..h5 | v h0..h5 (64 cols each head)
    wqkv = nc.dram_tensor("wqkv", [P, CSUB, 3 * D6], BF16, kind="ExternalInput")
    bqkv = nc.dram_tensor("bqkv", [P, QKV9], F32, kind="ExternalInput")
    wpj = nc.dram_tensor("wpj", [P, D6 // P, C], BF16, kind="ExternalInput")
    bpj = nc.dram_tensor("bpj", [P, CSUB], F32, kind="ExternalInput")
    # transposed causal mask (bf16): cmaskT[k, q] = 0 if k <= q else -1e30
    cmaskT = nc.dram_tensor("cmaskT", [P, P], BF16, kind="ExternalInput")
    out = nc.dram_tensor("attn_pT", [C, T], F32, kind="ExternalOutput")

    with tile.TileContext(nc) as tc:
        with (
            tc.tile_pool(name="const", bufs=1) as const,
            tc.tile_pool(name="xin", bufs=1) as xin,
            tc.tile_pool(name="big", bufs=1) as big,
            tc.tile_pool(name="pTp", bufs=2) as pTp,
            tc.tile_pool(name="work", bufs=3) as work,
            tc.tile_pool(name="ps", bufs=2, space="PSUM") as ps,
            tc.tile_pool(name="ps_t", bufs=1, space="PSUM") as ps_t,
            tc.tile_pool(name="ps_y", bufs=1, space="PSUM") as ps_y,
        ):
            # x tiles first: they gate the whole pipeline, so their DMAs must
            # not queue behind the (larger) weight loads
            xts = []
            for ti in range(NT):
                xt = xin.tile([P, C], F32, tag=f"x{ti}", name=f"x{ti}")
                nc.sync.dma_start(xt[:], xb[ti * P:(ti + 1) * P, :])
                xts.append(xt)

            ident = const.tile([P, P], BF16)
            make_identity(nc, ident[:])
            cm = const.tile([P, P], BF16)
            nc.sync.dma_start(cm[:], cmaskT[:])
            wqkv_sb = const.tile([P, CSUB, 3 * D6], BF16)
            nc.sync.dma_start(wqkv_sb[:], wqkv[:])
            bqkv_sb = const.tile([P, QKV9], F32)
            nc.sync.dma_start(bqkv_sb[:], bqkv[:])
            wpj_sb = const.tile([P, D6 // P, C], BF16)
            nc.sync.dma_start(wpj_sb[:], wpj[:])
            bpj_sb = const.tile([P, CSUB], F32)
            nc.sync.dma_start(bpj_sb[:], bpj[:])

            # ---- LN1 + transpose -> xlnT [C, T] (two T-half tiles so the
            # qkv matmuls can start after the first half)
            xlnT = [big.tile([P, CSUB, T // 2], BF16, tag=f"xlnT{i}",
                             name=f"xlnT{i}") for i in range(2)]
            for ti in range(NT):
                xt = xts[ti]
                s1 = work.tile([P, 1], F32, tag="s1")
                nc.vector.reduce_sum(s1[:], xt[:], axis=AX.X)
                sq = work.tile([P, C], F32, tag="sq")
                s2 = work.tile([P, 1], F32, tag="s2")
                nc.scalar.activation(sq[:], xt[:], AF.Square, accum_out=s2[:])
                negmu = work.tile([P, 1], F32, tag="negmu")
                nc.vector.tensor_scalar_mul(negmu[:], s1[:], -1.0 / C)
                # var = s2/C + eps - mu^2
                var = work.tile([P, 1], F32, tag="var")
                nc.vector.tensor_scalar(
                    var[:], s2[:], 1.0 / C, LN_EPS, op0=ALU.mult, op1=ALU.add)
                mu2 = work.tile([P, 1], F32, tag="mu2")
                nc.vector.tensor_tensor(mu2[:], negmu[:], negmu[:], op=ALU.mult)
                nc.vector.tensor_tensor(var[:], var[:], mu2[:], op=ALU.subtract)
                std = work.tile([P, 1], F32, tag="std")
                nc.scalar.activation(std[:], var[:], AF.Sqrt)
                rstd = work.tile([P, 1], F32, tag="rstd")
                nc.vector.reciprocal(rstd[:], std[:])
                xn = work.tile([P, C], BF16, tag="xn")
                nc.vector.tensor_scalar(
                    xn[:], xt[:], negmu[:], rstd[:],
                    op0=ALU.add, op1=ALU.mult)
                pt = ps_t.tile([P, CSUB, P], BF16, tag="pt6")
                for cs in range(CSUB):
                    nc.tensor.transpose(
                        pt[:, cs, :], xn[:, cs * P:(cs + 1) * P], ident[:])
                nc.vector.tensor_copy(
                    xlnT[ti // 4][:, :, (ti % 4) * P:(ti % 4 + 1) * P], pt[:])

            # ---- qkvT [3*D6, T] = wqkv.T @ xln.T, + bias
            # one SBUF tile per 128-row group so consumers wait only on the
            # rows they read, letting the head loop overlap this phase
            qkvT = [big.tile([P, T], BF16, tag=f"qkvT{mc}", name=f"qkvT{mc}")
                    for mc in range(QKV9)]
            for mc in range(QKV9):
                for th in range(T // 512):
                    pacc = ps.tile([P, 512], F32, tag="mm")
                    for ks in range(CSUB):
                        nc.tensor.matmul(
                            pacc[:],
                            lhsT=wqkv_sb[:, ks, mc * P:(mc + 1) * P],
                            rhs=xlnT[th][:, ks, :],
                            start=(ks == 0), stop=(ks == CSUB - 1))
                    nc.scalar.activation(
                        qkvT[mc][:, th * 512:(th + 1) * 512], pacc[:],
                        AF.Identity, bias=bqkv_sb[:, mc:mc + 1])

            # ---- v_ones [k_in, k_block, head, 1+64] from vT rows: col 0 is
            # all-ones (fused softmax-denominator column), cols 1..65 = v_h.
            v_ones = big.tile([P, NT, H6, 1 + 64], BF16)
            nc.vector.memset(v_ones[:, :, :, 0:1], 1.0)
            for ti in range(NT):
                pt6 = ps_t.tile([P, CSUB, P], BF16, tag="pt6")
                pt = pt6[:, :D6 // P, :]
                for j in range(D6 // P):
                    nc.tensor.transpose(
                        pt[:, j, :],
                        qkvT[2 * (D6 // P) + j][:, ti * P:(ti + 1) * P],
                        ident[:])
                nc.vector.tensor_copy(
                    v_ones[:, ti, :, 1:],
                    pt[:].rearrange("p j (a b) -> p (j a) b", a=2))

            # ---- attention: scores computed transposed, sT[k, q], so Exp
            # lands pT in SBUF directly; AV fuses the denominator via col 0.
            # The causal mask of the diagonal block is accumulated into PSUM
            # by the PE itself (I.T @ cmaskT).
            y_big = big.tile([P, NT, D6], BF16)
            for h in range(H6):
                qp0 = 64 * (h % 2)
                qrow = h // 2
                kp0 = (D6 + 64 * h) % P
                krow = (D6 + 64 * h) // P
                pT = pTp.tile([P, NT, T], BF16, tag="pT")
                for kb in range(NT):
                    q0 = kb * P
                    span = T - q0
                    pscore = ps.tile([P, T], F32, tag="sc")
                    # chunk on absolute 512 boundaries (PSUM bank alignment)
                    bounds = [q0] + [b for b in (512, T) if b > q0]
                    for (s0, e0) in zip(bounds[:-1], bounds[1:]):
                        w = e0 - s0
                        nc.tensor.matmul(
                            pscore[:, s0:s0 + w],
                            lhsT=qkvT[krow][kp0:kp0 + 64, kb * P:(kb + 1) * P],
                            rhs=qkvT[qrow][qp0:qp0 + 64, s0:s0 + w],
                            start=True, stop=True)
                    nc.tensor.matmul(
                        pscore[:, q0:q0 + P], lhsT=ident[:], rhs=cm[:],
                        start=False, stop=True, skip_group_check=True)
                    nc.scalar.activation(
                        pT[:, kb, q0:], pscore[:, q0:], AF.Exp)
                for qi in range(NT):
                    py = ps_y.tile([P, 65], F32)
                    for kb in range(qi + 1):
                        nc.tensor.matmul(
                            py[:], lhsT=pT[:, kb, qi * P:(qi + 1) * P],
                            rhs=v_ones[:, kb, h, :],
                            start=(kb == 0), stop=(kb == qi))
                    rec = work.tile([P, 1], F32, tag="rec")
                    nc.vector.reciprocal(rec[:], py[:, 0:1])
                    nc.vector.tensor_tensor(
                        y_big[:, qi, h * 64:(h + 1) * 64], py[:, 1:],
                        rec[:].to_broadcast([P, 64]), op=ALU.mult)

            # ---- yT [D6, T]
            yT = big.tile([P, D6 // P, T], BF16)
            for qi in range(NT):
                pt6 = ps_t.tile([P, CSUB, P], BF16, tag="pt6")
                pt = pt6[:, :D6 // P, :]
                for j in range(D6 // P):
                    nc.tensor.transpose(
                        pt[:, j, :], y_big[:, qi, j * P:(j + 1) * P], ident[:])
                nc.vector.tensor_copy(yT[:, :, qi * P:(qi + 1) * P], pt[:])

            # ---- partial projection: outT [C, T] = wpj.T @ y.T + bpj
            for cc in range(CSUB):
                pacc = ps.tile([P, T], F32, tag="sc")
                o_sb = work.tile([P, T], F32, tag="osb")
                for th in range(T // 512):
                    for j in range(D6 // P):
                        nc.tensor.matmul(
                            pacc[:, th * 512:(th + 1) * 512],
                            lhsT=wpj_sb[:, j, cc * P:(cc + 1) * P],
                            rhs=yT[:, j, th * 512:(th + 1) * 512],
                            start=(j == 0), stop=(j == D6 // P - 1))
                nc.scalar.activation(
                    o_sb[:], pacc[:], AF.Identity, bias=bpj_sb[:, cc:cc + 1])
                nc.sync.dma_start(out[cc * P:(cc + 1) * P, :], o_sb[:])

    nc.compile()
    return nc


# --------------------------------------------------------------------------
# Launch B: experts
# --------------------------------------------------------------------------

def build_expert(cap_k):
    nc = bacc.Bacc("TRN2", target_bir_lowering=False, debug=False)

    xbT = nc.dram_tensor("xbT", [P, CSUB, cap_k], BF16, kind="ExternalInput")
    fcw = nc.dram_tensor("fcw", [KSUB_F, P, CSUB, P], BF16,
                         kind="ExternalInput")
    fcb = nc.dram_tensor("fcb", [P, KSUB_F], F32, kind="ExternalInput")
    pjw = nc.dram_tensor("pjw", [CSUB, P, KSUB_F, P], BF16,
                         kind="ExternalInput")
    pjb = nc.dram_tensor("pjb", [P, CSUB], F32, kind="ExternalInput")
    out = nc.dram_tensor("outT", [C, cap_k], F32, kind="ExternalOutput")

    SC = _chunks(cap_k)

    with tile.TileContext(nc) as tc:
        with (
            tc.tile_pool(name="const", bufs=1) as const,
            tc.tile_pool(name="w1", bufs=6) as w1p,
            tc.tile_pool(name="w2", bufs=4) as w2p,
            tc.tile_pool(name="big", bufs=1) as big,
            tc.tile_pool(name="osb", bufs=2) as osbp,
            tc.tile_pool(name="ps", bufs=4, space="PSUM") as ps,
        ):
            xbT_sb = const.tile([P, CSUB, cap_k], BF16)
            nc.sync.dma_start(xbT_sb[:], xbT[:])
            fcb_sb = const.tile([P, KSUB_F], F32)
            nc.sync.dma_start(fcb_sb[:], fcb[:])
            pjb_sb = const.tile([P, CSUB], F32)
            nc.sync.dma_start(pjb_sb[:], pjb[:])

            hT = big.tile([P, KSUB_F, cap_k], BF16)
            for mf in range(KSUB_F):
                wt = w1p.tile([P, CSUB, P], BF16, tag="w1")
                nc.sync.dma_start(wt[:], fcw[mf])
                for (s0, sw) in SC:
                    pacc = ps.tile([P, 512], F32, tag="mm")
                    for ks in range(CSUB):
                        nc.tensor.matmul(
                            pacc[:, :sw], lhsT=wt[:, ks, :],
                            rhs=xbT_sb[:, ks, s0:s0 + sw],
                            start=(ks == 0), stop=(ks == CSUB - 1))
                    nc.scalar.activation(
                        hT[:, mf, s0:s0 + sw], pacc[:, :sw],
                        AF.Gelu, bias=fcb_sb[:, mf:mf + 1])

            for cc in range(CSUB):
                wt = w2p.tile([P, KSUB_F, P], BF16, tag="w2")
                nc.sync.dma_start(wt[:], pjw[cc])
                o_sb = osbp.tile([P, cap_k], F32, tag="osb")
                for (s0, sw) in SC:
                    pacc = ps.tile([P, 512], F32, tag="mm")
                    for ks in range(KSUB_F):
                        nc.tensor.matmul(
                            pacc[:, :sw], lhsT=wt[:, ks, :],
                            rhs=hT[:, ks, s0:s0 + sw],
                            start=(ks == 0), stop=(ks == KSUB_F - 1))
                    nc.scalar.activation(
                        o_sb[:, s0:s0 + sw], pacc[:, :sw],
                        AF.Identity, bias=pjb_sb[:, cc:cc + 1])
                nc.sync.dma_start(out[cc * P:(cc + 1) * P, :], o_sb[:])

    nc.compile()
    return nc


# --------------------------------------------------------------------------
# Host glue
# --------------------------------------------------------------------------

def _bf16(a):
    return np.asarray(a, np.float32).astype(ml_dtypes.bfloat16)


def _pcol(vec, nsub):
    """[nsub*P] -> [P, nsub] per-partition bias layout."""
    return np.ascontiguousarray(
        np.asarray(vec, np.float32).reshape(nsub, P).T)


def _kperm(w):
    """[K, N] -> [P, K//P, N] partition-major layout, contiguous."""
    k, n = w.shape
    return np.ascontiguousarray(w.reshape(k // P, P, n).transpose(1, 0, 2))


def _layer_norm(x, w, b):
    mu = x.mean(-1, keepdims=True)
    var = x.var(-1, keepdims=True)
    return (x - mu) / np.sqrt(var + LN_EPS) * w + b


def _exact_logits(need, x, ln1_w, ln1_b, ln2_w, ln2_b, qkv_w, qkv_b,
                  proj_w, proj_b, w_g):
    """fp32 gating logits for the given flat token indices (exact attention
    rows for just those tokens)."""
    out = np.empty((need.size, E), np.float32)
    bs, ps = need // T, need % T
    for b in np.unique(bs):
        m = bs == b
        pos = ps[m]                              # [M]
        xl = _layer_norm(x[b], ln1_w, ln1_b)     # [T, C]
        kv = xl @ qkv_w[:, C:] + qkv_b[C:]       # [T, 2C]
        k = kv[:, :C].reshape(T, NHEAD, HD)
        v = kv[:, C:].reshape(T, NHEAD, HD)
        q = (xl[pos] @ qkv_w[:, :C] + qkv_b[:C]).reshape(-1, NHEAD, HD)
        s = np.einsum("mhd,khd->mhk", q, k) / math.sqrt(HD)
        s = np.where(pos[:, None, None] >= np.arange(T)[None, None, :],
                     s, NEG_INF)
        s -= s.max(-1, keepdims=True)
        p = np.exp(s)
        p /= p.sum(-1, keepdims=True)
        y = np.einsum("mhk,khd->mhd", p, v).reshape(-1, C)
        att = y @ proj_w + proj_b
        x2 = x[b][pos] + att
        out[m] = _layer_norm(x2, ln2_w, ln2_b) @ w_g
    return out


def kernel(x, ln1_w, ln1_b, ln2_w, ln2_b, attn_qkv_w, attn_qkv_b,
           attn_proj_w, attn_proj_b, w_g, exp_fc_w, exp_fc_b,
           exp_proj_w, exp_proj_b):
    x = np.asarray(x, np.float32)
    ln1_w = np.asarray(ln1_w, np.float32)
    ln1_b = np.asarray(ln1_b, np.float32)
    attn_qkv_w = np.asarray(attn_qkv_w, np.float32)
    attn_qkv_b = np.asarray(attn_qkv_b, np.float32)
    attn_proj_w = np.asarray(attn_proj_w, np.float32)
    attn_proj_b = np.asarray(attn_proj_b, np.float32)

    if "attn" not in _CACHE:
        _CACHE["attn"] = build_attn()

    # ---------------- launch A ----------------
    # fold ln1 affine into qkv: qkv = xhat @ (diag(w1) W) + (b1 @ W + b)
    Wf = ln1_w[:, None] * attn_qkv_w          # [C, 3C]
    bf = ln1_b @ attn_qkv_w + attn_qkv_b      # [3C]
    Wq = Wf[:, :C] / math.sqrt(HD)
    bq = bf[:C] / math.sqrt(HD)
    Wk, bk = Wf[:, C:2 * C], bf[C:2 * C]
    Wv, bv = Wf[:, 2 * C:], bf[2 * C:]

    cmaskT_np = _bf16(np.where(
        np.triu(np.ones((P, P), bool)), 0.0, NEG_INF))

    in_maps_a = []
    for core in range(N_CORES):
        b = core // 2
        h0 = H6 * (core % 2)
        cols = slice(h0 * HD, (h0 + H6) * HD)
        wqkv_c = np.concatenate([Wq[:, cols], Wk[:, cols], Wv[:, cols]], 1)
        bqkv_c = np.concatenate([bq[cols], bk[cols], bv[cols]])
        bpj_c = attn_proj_b if core % 2 == 0 else np.zeros(C, np.float32)
        in_maps_a.append({
            "xb": np.ascontiguousarray(x[b]),
            "wqkv": _kperm(_bf16(wqkv_c)),
            "bqkv": _pcol(bqkv_c, QKV9),
            "wpj": _kperm(_bf16(attn_proj_w[h0 * HD:(h0 + H6) * HD, :])),
            "bpj": _pcol(bpj_c, CSUB),
            "cmaskT": cmaskT_np,
        })

    res_a = bass_utils.run_bass_kernel_spmd(
        _CACHE["attn"], in_maps_a, core_ids=list(range(N_CORES)))

    attn = np.empty((B, T, C), np.float32)
    for b in range(B):
        attn[b] = (res_a.results[2 * b]["attn_pT"]
                   + res_a.results[2 * b + 1]["attn_pT"]).T

    x2 = x + attn                       # [B, T, C]
    xf2 = x2.reshape(B * T, C)

    # ---------------- host routing (exact reference semantics) -------------
    N = B * T
    xln2 = _layer_norm(xf2, np.asarray(ln2_w, np.float32),
                       np.asarray(ln2_b, np.float32))
    logits = xln2 @ np.asarray(w_g, np.float32)        # [N, E]

    # The top-2 expert choice is discontinuous: tokens whose top2/top3 gating
    # logits are within the bf16 noise floor could route differently than the
    # fp32 reference would. Recompute those few tokens' logits exactly.
    srt = np.sort(logits, axis=1)
    need = np.nonzero(srt[:, -2] - srt[:, -3] < 0.02)[0]
    if need.size:
        logits[need] = _exact_logits(
            need, x, ln1_w, ln1_b, np.asarray(ln2_w, np.float32),
            np.asarray(ln2_b, np.float32), attn_qkv_w, attn_qkv_b,
            attn_proj_w, attn_proj_b, np.asarray(w_g, np.float32))

    order = np.argsort(-logits, axis=1, kind="stable")
    topk_idx = order[:, :TOPK]                          # [N, K]
    sel = np.zeros((N, E), bool)
    np.put_along_axis(sel, topk_idx, True, axis=1)
    masked = np.where(sel, logits, NEG_INF)
    m = masked.max(1, keepdims=True)
    ex = np.exp(masked - m)
    router_probs = ex / ex.sum(1, keepdims=True)        # [N, E]

    # capacity ranks in (k, n) order
    exp_mask = np.zeros((TOPK, N, E), np.int64)
    kk = np.arange(TOPK)[:, None]
    nn = np.arange(N)[None, :]
    exp_mask[kk, nn, topk_idx.T] = 1
    flat = exp_mask.reshape(TOPK * N, E)
    rank = np.cumsum(flat, axis=0) - 1                  # [K*N, E]
    keep = (flat == 1) & (rank < CAP)
    kpos, epos = np.nonzero(keep)
    token = kpos % N
    slot = rank[kpos, epos]
    wgt = router_probs[token, epos]

    # pack the expert batches to the observed max load; if only a few rows
    # push one expert past 1024 slots (= 2 full PSUM chunks), keep the device
    # batch at 1024 and run the leftover rows on the host in fp32.
    loads = np.bincount(epos, minlength=E)
    max_load = int(loads.max())
    cap_k64 = max(64, -(-max_load // 64) * 64)
    overflow = int(np.maximum(loads - 1024, 0).sum())
    cap_k = 1024 if (cap_k64 > 1024 and overflow <= 192) \
        else min(CAP, cap_k64)
    if ("expert", cap_k) not in _CACHE:
        _CACHE[("expert", cap_k)] = build_expert(cap_k)

    on_dev = slot < cap_k
    idx_e = np.zeros((E, cap_k), np.int64)
    w_e = np.zeros((E, cap_k), np.float32)
    idx_e[epos[on_dev], slot[on_dev]] = token[on_dev]
    w_e[epos[on_dev], slot[on_dev]] = wgt[on_dev]

    # ---------------- launch B ----------------
    xln2_bf = _bf16(xln2)
    exp_fc_w = np.asarray(exp_fc_w, np.float32)
    exp_fc_b = np.asarray(exp_fc_b, np.float32).reshape(E, F)
    exp_proj_w = np.asarray(exp_proj_w, np.float32)
    exp_proj_b = np.asarray(exp_proj_b, np.float32).reshape(E, C)

    in_maps_b = []
    for e in range(E):
        xbT = _kperm(np.ascontiguousarray(xln2_bf[idx_e[e]].T))
        fcw = _bf16(exp_fc_w[e]).reshape(CSUB, P, KSUB_F, P)
        fcw = np.ascontiguousarray(fcw.transpose(2, 1, 0, 3))
        pjw = _bf16(exp_proj_w[e]).reshape(KSUB_F, P, CSUB, P)
        pjw = np.ascontiguousarray(pjw.transpose(2, 1, 0, 3))
        in_maps_b.append({
            "xbT": xbT,
            "fcw": fcw,
            "fcb": _pcol(exp_fc_b[e], KSUB_F),
            "pjw": pjw,
            "pjb": _pcol(exp_proj_b[e], CSUB),
        })

    res_b = bass_utils.run_bass_kernel_spmd(
        _CACHE[("expert", cap_k)], in_maps_b, core_ids=list(range(N_CORES)))

    y = xf2.copy()
    for e in range(E):
        valid = w_e[e] != 0
        y[idx_e[e, valid]] += (w_e[e, valid, None]
                               * res_b.results[e]["outT"].T[valid])

    # host top-up for the few rows beyond cap_k (exact fp32)
    if not on_dev.all():
        try:
            from scipy.special import erf
        except ImportError:
            erf = np.vectorize(math.erf)
        off = ~on_dev
        for e in np.unique(epos[off]):
            m = off & (epos == e)
            tk = token[m]
            h = xln2[tk] @ exp_fc_w[e] + exp_fc_b[e]
            h = 0.5 * h * (1.0 + erf(h / math.sqrt(2.0)))
            o = h @ exp_proj_w[e] + exp_proj_b[e]
            y[tk] += wgt[m, None] * o
    return y.reshape(B, T, C).astype(np.float32)


# revision 29
# speedup vs baseline: 1.2114x; 1.0199x over previous
"""MoE transformer block on 8 Trainium2 cores.

Layer: x = x + attn(ln1(x)); x = x + moe(ln2(x)).
Shapes: B=4, T=1024, C=768, H=12 heads, E=8 experts, top-2, cap=1280, F=3072.

Distribution:
  Launch A (attention): core i -> batch i//2, heads 6*(i%2) .. +6.
    LN1 affine is folded into the QKV weights host-side; each core emits a
    partial (6-head) output projection, transposed [C, T], f32. Host sums the
    two half-head partials per batch and adds the residual.
  Host: ln2 + gating + exact top-2 capacity routing (numpy, matches the jax
    reference in ordering; near-tie tokens get exact fp32 logits), builds
    per-expert gather indices.
  Launch B (experts): core e -> expert e, slots packed to the observed max
    load (rounded up to 64). xbT [C, cap_k] bf16 in, outT [C, cap_k] f32 out.
    Host scatter-adds w * out into y (per-expert indices are unique, so
    fancy-index += is collision-free).
"""

import math

import numpy as np
import ml_dtypes

import concourse.bacc as bacc
import concourse.bass as bass
import concourse.mybir as mybir
import concourse.tile as tile
from concourse import bass_utils
from concourse.masks import make_identity

F32 = mybir.dt.float32
BF16 = mybir.dt.bfloat16
AF = mybir.ActivationFunctionType
ALU = mybir.AluOpType
AX = mybir.AxisListType

B, T, C = 4, 1024, 768
NHEAD = 12
HD = C // NHEAD  # 64
E = 8
TOPK = 2
CAP = 1280
F = 4 * C  # 3072
LN_EPS = 1e-5
NEG_INF = -1e30
P = 128

N_CORES = 8
H6 = NHEAD // 2          # heads per core
D6 = H6 * HD             # 384
CSUB = C // P            # 6
KSUB_F = F // P          # 24
NT = T // P              # 8
QKV9 = 3 * D6 // P       # 9

_CACHE = {}


def _chunks(n, step=512):
    out = []
    s = 0
    while s < n:
        out.append((s, min(step, n - s)))
        s += step
    return out


# --------------------------------------------------------------------------
# Launch A: attention
# --------------------------------------------------------------------------

def build_attn():
    nc = bacc.Bacc("TRN2", target_bir_lowering=False, debug=False)

    xb = nc.dram_tensor("xb", [T, C], F32, kind="ExternalInput")
    # qkv weight slice for this core's 6 heads, ln1-folded, q pre-scaled by
    # 1/sqrt(HD), pre-permuted to [p, ks, n]. column order within n:
    # q h0..h5 | k h0..h5 | v h0..h5 (64 cols each head)
    wqkv = nc.dram_tensor("wqkv", [P, CSUB, 3 * D6], BF16, kind="ExternalInput")
    bqkv = nc.dram_tensor("bqkv", [P, QKV9], F32, kind="ExternalInput")
    wpj = nc.dram_tensor("wpj", [P, D6 // P, C], BF16, kind="ExternalInput")
    bpj = nc.dram_tensor("bpj", [P, CSUB], F32, kind="ExternalInput")
    # transposed causal mask (bf16): cmaskT[k, q] = 0 if k <= q else -1e30
    cmaskT = nc.dram_tensor("cmaskT", [P, P], BF16, kind="ExternalInput")
    out = nc.dram_tensor("attn_pT", [C, T], F32, kind="ExternalOutput")

    with tile.TileContext(nc) as tc:
        with (
            tc.tile_pool(name="const", bufs=1) as const,
            tc.tile_pool(name="xin", bufs=1) as xin,
            tc.tile_pool(name="big", bufs=1) as big,
            tc.tile_pool(name="pTp", bufs=2) as pTp,
            tc.tile_pool(name="work", bufs=3) as work,
            tc.tile_pool(name="ps", bufs=2, space="PSUM") as ps,
            tc.tile_pool(name="ps_t", bufs=1, space="PSUM") as ps_t,
            tc.tile_pool(name="ps_y", bufs=1, space="PSUM") as ps_y,
        ):
            # x tiles first: they gate the whole pipeline, so their DMAs must
            # not queue behind the (larger) weight loads
            xts = []
            for ti in range(NT):
                xt = xin.tile([P, C], F32, tag=f"x{ti}", name=f"x{ti}")
                nc.sync.dma_start(xt[:], xb[ti * P:(ti + 1) * P, :])
                xts.append(xt)

            ident = const.tile([P, P], BF16)
            make_identity(nc, ident[:])
            cm = const.tile([P, P], BF16)
            nc.sync.dma_start(cm[:], cmaskT[:])
            wqkv_sb = const.tile([P, CSUB, 3 * D6], BF16)
            nc.sync.dma_start(wqkv_sb[:], wqkv[:])
            bqkv_sb = const.tile([P, QKV9], F32)
            nc.sync.dma_start(bqkv_sb[:], bqkv[:])
            wpj_sb = const.tile([P, D6 // P, C], BF16)
            nc.sync.dma_start(wpj_sb[:], wpj[:])
            bpj_sb = const.tile([P, CSUB], F32)
            nc.sync.dma_start(bpj_sb[:], bpj[:])

            # ---- LN1 + transpose -> xlnT [C, T] (two T-half tiles so the
            # qkv matmuls can start after the first half)
            xlnT = [big.tile([P, CSUB, T // 2], BF16, tag=f"xlnT{i}",
                             name=f"xlnT{i}") for i in range(2)]
            for ti in range(NT):
                xt = xts[ti]
                s1 = work.tile([P, 1], F32, tag="s1")
                nc.vector.reduce_sum(s1[:], xt[:], axis=AX.X)
                sq = work.tile([P, C], F32, tag="sq")
                s2 = work.tile([P, 1], F32, tag="s2")
                nc.scalar.activation(sq[:], xt[:], AF.Square, accum_out=s2[:])
                negmu = work.tile([P, 1], F32, tag="negmu")
                nc.vector.tensor_scalar_mul(negmu[:], s1[:], -1.0 / C)
                # var = s2/C + eps - mu^2
                var = work.tile([P, 1], F32, tag="var")
                nc.vector.tensor_scalar(
                    var[:], s2[:], 1.0 / C, LN_EPS, op0=ALU.mult, op1=ALU.add)
                mu2 = work.tile([P, 1], F32, tag="mu2")
                nc.vector.tensor_tensor(mu2[:], negmu[:], negmu[:], op=ALU.mult)
                nc.vector.tensor_tensor(var[:], var[:], mu2[:], op=ALU.subtract)
                std = work.tile([P, 1], F32, tag="std")
                nc.scalar.activation(std[:], var[:], AF.Sqrt)
                rstd = work.tile([P, 1], F32, tag="rstd")
                nc.vector.reciprocal(rstd[:], std[:])
                xn = work.tile([P, C], BF16, tag="xn")
                nc.gpsimd.tensor_scalar(
                    xn[:], xt[:], negmu[:], rstd[:],
                    op0=ALU.add, op1=ALU.mult)
                pt = ps_t.tile([P, CSUB, P], BF16, tag="pt6")
                for cs in range(CSUB):
                    nc.tensor.transpose(
                        pt[:, cs, :], xn[:, cs * P:(cs + 1) * P], ident[:])
                nc.vector.tensor_copy(
                    xlnT[ti // 4][:, :, (ti % 4) * P:(ti % 4 + 1) * P], pt[:])

            # ---- qkvT [3*D6, T] = wqkv.T @ xln.T, + bias
            # one SBUF tile per 128-row group so consumers wait only on the
            # rows they read, letting the head loop overlap this phase
            qkvT = [big.tile([P, T], BF16, tag=f"qkvT{mc}", name=f"qkvT{mc}")
                    for mc in range(QKV9)]
            v_ones = big.tile([P, NT, H6, 1 + 64], BF16)
            nc.vector.memset(v_ones[:, :, :, 0:1], 1.0)
            y_big = big.tile([P, NT, D6], BF16)

            def emit_qkv(mc):
                for th in range(T // 512):
                    pacc = ps.tile([P, 512], F32, tag="mm", name=f"qk{mc}{th}")
                    for ks in range(CSUB):
                        nc.tensor.matmul(
                            pacc[:],
                            lhsT=wqkv_sb[:, ks, mc * P:(mc + 1) * P],
                            rhs=xlnT[th][:, ks, :],
                            start=(ks == 0), stop=(ks == CSUB - 1))
                    nc.scalar.activation(
                        qkvT[mc][:, th * 512:(th + 1) * 512], pacc[:],
                        AF.Identity, bias=bqkv_sb[:, mc:mc + 1])

            def emit_vones(j):
                # vT row j -> v for heads 2j, 2j+1 (col 0 stays all-ones)
                for ti in range(NT):
                    pt6 = ps_t.tile([P, CSUB, P], BF16, tag="pt6",
                                    name=f"vt{j}{ti}")
                    nc.tensor.transpose(
                        pt6[:, 0, :],
                        qkvT[2 * (D6 // P) + j][:, ti * P:(ti + 1) * P],
                        ident[:])
                    nc.vector.tensor_copy(
                        v_ones[:, ti, 2 * j:2 * j + 2, 1:],
                        pt6[:, 0, :].rearrange("p (a b) -> p a b", a=2))

            def emit_head(h):
                # scores transposed sT[k, q] so Exp lands pT in SBUF directly;
                # AV fuses the softmax denominator via v_ones col 0; the
                # causal mask of the diagonal block is added by the PE itself.
                qp0 = 64 * (h % 2)
                qrow = h // 2
                kp0 = (D6 + 64 * h) % P
                krow = (D6 + 64 * h) // P
                pT = pTp.tile([P, NT, T], BF16, tag="pT", name=f"pT{h}")
                for kb in range(NT):
                    q0 = kb * P
                    pscore = ps.tile([P, T], F32, tag="sc", name=f"sc{h}{kb}")
                    # chunk on absolute 512 boundaries (PSUM bank alignment)
                    bounds = [q0] + [b for b in (512, T) if b > q0]
                    for (s0, e0) in zip(bounds[:-1], bounds[1:]):
                        w = e0 - s0
                        nc.tensor.matmul(
                            pscore[:, s0:s0 + w],
                            lhsT=qkvT[krow][kp0:kp0 + 64, kb * P:(kb + 1) * P],
                            rhs=qkvT[qrow][qp0:qp0 + 64, s0:s0 + w],
                            start=True, stop=True)
                    nc.tensor.matmul(
                        pscore[:, q0:q0 + P], lhsT=ident[:], rhs=cm[:],
                        start=False, stop=True, skip_group_check=True)
                    nc.scalar.activation(
                        pT[:, kb, q0:], pscore[:, q0:], AF.Exp)
                for qi in range(NT):
                    py = ps_y.tile([P, 65], F32, tag="py", name=f"py{h}{qi}")
                    for kb in range(qi + 1):
                        nc.tensor.matmul(
                            py[:], lhsT=pT[:, kb, qi * P:(qi + 1) * P],
                            rhs=v_ones[:, kb, h, :],
                            start=(kb == 0), stop=(kb == qi))
                    rec = work.tile([P, 1], F32, tag="rec")
                    nc.vector.reciprocal(rec[:], py[:, 0:1])
                    nc.vector.tensor_tensor(
                        y_big[:, qi, h * 64:(h + 1) * 64], py[:, 1:],
                        rec[:].to_broadcast([P, 64]), op=ALU.mult)

            # interleave: emit each head-pair's q/k/v columns, its v
            # transposes, then its two heads, so ACT's exp work starts while
            # the PE is still on later qkv matmuls
            for g in range(D6 // P):
                emit_qkv(g)
                emit_qkv(3 + g)
                emit_qkv(6 + g)
                emit_vones(g)
                emit_head(2 * g)
                emit_head(2 * g + 1)

            # ---- yT [D6, T]
            yT = big.tile([P, D6 // P, T], BF16)
            for qi in range(NT):
                pt6 = ps_t.tile([P, CSUB, P], BF16, tag="pt6")
                pt = pt6[:, :D6 // P, :]
                for j in range(D6 // P):
                    nc.tensor.transpose(
                        pt[:, j, :], y_big[:, qi, j * P:(j + 1) * P], ident[:])
                nc.vector.tensor_copy(yT[:, :, qi * P:(qi + 1) * P], pt[:])

            # ---- partial projection: outT [C, T] = wpj.T @ y.T + bpj
            for cc in range(CSUB):
                pacc = ps.tile([P, T], F32, tag="sc")
                o_sb = work.tile([P, T], F32, tag="osb")
                for th in range(T // 512):
                    for j in range(D6 // P):
                        nc.tensor.matmul(
                            pacc[:, th * 512:(th + 1) * 512],
                            lhsT=wpj_sb[:, j, cc * P:(cc + 1) * P],
                            rhs=yT[:, j, th * 512:(th + 1) * 512],
                            start=(j == 0), stop=(j == D6 // P - 1))
                nc.scalar.activation(
                    o_sb[:], pacc[:], AF.Identity, bias=bpj_sb[:, cc:cc + 1])
                nc.sync.dma_start(out[cc * P:(cc + 1) * P, :], o_sb[:])

    nc.compile()
    return nc


# --------------------------------------------------------------------------
# Launch B: experts
# --------------------------------------------------------------------------

def build_expert(cap_k):
    nc = bacc.Bacc("TRN2", target_bir_lowering=False, debug=False)

    xbT = nc.dram_tensor("xbT", [P, CSUB, cap_k], BF16, kind="ExternalInput")
    fcw = nc.dram_tensor("fcw", [KSUB_F, P, CSUB, P], BF16,
                         kind="ExternalInput")
    fcb = nc.dram_tensor("fcb", [P, KSUB_F], F32, kind="ExternalInput")
    pjw = nc.dram_tensor("pjw", [CSUB, P, KSUB_F, P], BF16,
                         kind="ExternalInput")
    pjb = nc.dram_tensor("pjb", [P, CSUB], F32, kind="ExternalInput")
    out = nc.dram_tensor("outT", [C, cap_k], F32, kind="ExternalOutput")

    SC = _chunks(cap_k)

    with tile.TileContext(nc) as tc:
        with (
            tc.tile_pool(name="const", bufs=1) as const,
            tc.tile_pool(name="w1", bufs=6) as w1p,
            tc.tile_pool(name="w2", bufs=4) as w2p,
            tc.tile_pool(name="big", bufs=1) as big,
            tc.tile_pool(name="osb", bufs=2) as osbp,
            tc.tile_pool(name="ps", bufs=4, space="PSUM") as ps,
        ):
            xbT_sb = const.tile([P, CSUB, cap_k], BF16)
            for (s0, sw) in SC:
                nc.sync.dma_start(
                    xbT_sb[:, :, s0:s0 + sw], xbT[:, :, s0:s0 + sw])
            fcb_sb = const.tile([P, KSUB_F], F32)
            nc.sync.dma_start(fcb_sb[:], fcb[:])
            pjb_sb = const.tile([P, CSUB], F32)
            nc.sync.dma_start(pjb_sb[:], pjb[:])

            hT = big.tile([P, KSUB_F, cap_k], BF16)
            for mf in range(KSUB_F):
                wt = w1p.tile([P, CSUB, P], BF16, tag="w1")
                nc.sync.dma_start(wt[:], fcw[mf])
                for (s0, sw) in SC:
                    pacc = ps.tile([P, 512], F32, tag="mm")
                    for ks in range(CSUB):
                        nc.tensor.matmul(
                            pacc[:, :sw], lhsT=wt[:, ks, :],
                            rhs=xbT_sb[:, ks, s0:s0 + sw],
                            start=(ks == 0), stop=(ks == CSUB - 1))
                    nc.scalar.activation(
                        hT[:, mf, s0:s0 + sw], pacc[:, :sw],
                        AF.Gelu, bias=fcb_sb[:, mf:mf + 1])

            for cc in range(CSUB):
                wt = w2p.tile([P, KSUB_F, P], BF16, tag="w2")
                nc.sync.dma_start(wt[:], pjw[cc])
                o_sb = osbp.tile([P, cap_k], F32, tag="osb")
                for (s0, sw) in SC:
                    pacc = ps.tile([P, 512], F32, tag="mm")
                    for ks in range(KSUB_F):
                        nc.tensor.matmul(
                            pacc[:, :sw], lhsT=wt[:, ks, :],
                            rhs=hT[:, ks, s0:s0 + sw],
                            start=(ks == 0), stop=(ks == KSUB_F - 1))
                    nc.scalar.activation(
                        o_sb[:, s0:s0 + sw], pacc[:, :sw],
                        AF.Identity, bias=pjb_sb[:, cc:cc + 1])
                nc.sync.dma_start(out[cc * P:(cc + 1) * P, :], o_sb[:])

    nc.compile()
    return nc


# --------------------------------------------------------------------------
# Host glue
# --------------------------------------------------------------------------

def _bf16(a):
    return np.asarray(a, np.float32).astype(ml_dtypes.bfloat16)


def _pcol(vec, nsub):
    """[nsub*P] -> [P, nsub] per-partition bias layout."""
    return np.ascontiguousarray(
        np.asarray(vec, np.float32).reshape(nsub, P).T)


def _kperm(w):
    """[K, N] -> [P, K//P, N] partition-major layout, contiguous."""
    k, n = w.shape
    return np.ascontiguousarray(w.reshape(k // P, P, n).transpose(1, 0, 2))


def _layer_norm(x, w, b):
    mu = x.mean(-1, keepdims=True)
    var = x.var(-1, keepdims=True)
    return (x - mu) / np.sqrt(var + LN_EPS) * w + b


def _exact_logits(need, x, ln1_w, ln1_b, ln2_w, ln2_b, qkv_w, qkv_b,
                  proj_w, proj_b, w_g):
    """fp32 gating logits for the given flat token indices (exact attention
    rows for just those tokens)."""
    out = np.empty((need.size, E), np.float32)
    bs, ps = need // T, need % T
    for b in np.unique(bs):
        m = bs == b
        pos = ps[m]                              # [M]
        xl = _layer_norm(x[b], ln1_w, ln1_b)     # [T, C]
        kv = xl @ qkv_w[:, C:] + qkv_b[C:]       # [T, 2C]
        k = kv[:, :C].reshape(T, NHEAD, HD)
        v = kv[:, C:].reshape(T, NHEAD, HD)
        q = (xl[pos] @ qkv_w[:, :C] + qkv_b[:C]).reshape(-1, NHEAD, HD)
        s = np.einsum("mhd,khd->mhk", q, k) / math.sqrt(HD)
        s = np.where(pos[:, None, None] >= np.arange(T)[None, None, :],
                     s, NEG_INF)
        s -= s.max(-1, keepdims=True)
        p = np.exp(s)
        p /= p.sum(-1, keepdims=True)
        y = np.einsum("mhk,khd->mhd", p, v).reshape(-1, C)
        att = y @ proj_w + proj_b
        x2 = x[b][pos] + att
        out[m] = _layer_norm(x2, ln2_w, ln2_b) @ w_g
    return out


def kernel(x, ln1_w, ln1_b, ln2_w, ln2_b, attn_qkv_w, attn_qkv_b,
           attn_proj_w, attn_proj_b, w_g, exp_fc_w, exp_fc_b,
           exp_proj_w, exp_proj_b):
    x = np.asarray(x, np.float32)
    ln1_w = np.asarray(ln1_w, np.float32)
    ln1_b = np.asarray(ln1_b, np.float32)
    attn_qkv_w = np.asarray(attn_qkv_w, np.float32)
    attn_qkv_b = np.asarray(attn_qkv_b, np.float32)
    attn_proj_w = np.asarray(attn_proj_w, np.float32)
    attn_proj_b = np.asarray(attn_proj_b, np.float32)

    if "attn" not in _CACHE:
        _CACHE["attn"] = build_attn()

    # ---------------- launch A ----------------
    # fold ln1 affine into qkv: qkv = xhat @ (diag(w1) W) + (b1 @ W + b)
    Wf = ln1_w[:, None] * attn_qkv_w          # [C, 3C]
    bf = ln1_b @ attn_qkv_w + attn_qkv_b      # [3C]
    Wq = Wf[:, :C] / math.sqrt(HD)
    bq = bf[:C] / math.sqrt(HD)
    Wk, bk = Wf[:, C:2 * C], bf[C:2 * C]
    Wv, bv = Wf[:, 2 * C:], bf[2 * C:]

    cmaskT_np = _bf16(np.where(
        np.triu(np.ones((P, P), bool)), 0.0, NEG_INF))

    in_maps_a = []
    for core in range(N_CORES):
        b = core // 2
        h0 = H6 * (core % 2)
        cols = slice(h0 * HD, (h0 + H6) * HD)
        wqkv_c = np.concatenate([Wq[:, cols], Wk[:, cols], Wv[:, cols]], 1)
        bqkv_c = np.concatenate([bq[cols], bk[cols], bv[cols]])
        bpj_c = attn_proj_b if core % 2 == 0 else np.zeros(C, np.float32)
        in_maps_a.append({
            "xb": np.ascontiguousarray(x[b]),
            "wqkv": _kperm(_bf16(wqkv_c)),
            "bqkv": _pcol(bqkv_c, QKV9),
            "wpj": _kperm(_bf16(attn_proj_w[h0 * HD:(h0 + H6) * HD, :])),
            "bpj": _pcol(bpj_c, CSUB),
            "cmaskT": cmaskT_np,
        })

    res_a = bass_utils.run_bass_kernel_spmd(
        _CACHE["attn"], in_maps_a, core_ids=list(range(N_CORES)))

    attn = np.empty((B, T, C), np.float32)
    for b in range(B):
        attn[b] = (res_a.results[2 * b]["attn_pT"]
                   + res_a.results[2 * b + 1]["attn_pT"]).T

    x2 = x + attn                       # [B, T, C]
    xf2 = x2.reshape(B * T, C)

    # ---------------- host routing (exact reference semantics) -------------
    N = B * T
    xln2 = _layer_norm(xf2, np.asarray(ln2_w, np.float32),
                       np.asarray(ln2_b, np.float32))
    logits = xln2 @ np.asarray(w_g, np.float32)        # [N, E]

    # The top-2 expert choice is discontinuous: tokens whose top2/top3 gating
    # logits are within the bf16 noise floor could route differently than the
    # fp32 reference would. Recompute those few tokens' logits exactly.
    srt = np.sort(logits, axis=1)
    need = np.nonzero(srt[:, -2] - srt[:, -3] < 0.02)[0]
    if need.size:
        logits[need] = _exact_logits(
            need, x, ln1_w, ln1_b, np.asarray(ln2_w, np.float32),
            np.asarray(ln2_b, np.float32), attn_qkv_w, attn_qkv_b,
            attn_proj_w, attn_proj_b, np.asarray(w_g, np.float32))

    order = np.argsort(-logits, axis=1, kind="stable")
    topk_idx = order[:, :TOPK]                          # [N, K]
    sel = np.zeros((N, E), bool)
    np.put_along_axis(sel, topk_idx, True, axis=1)
    masked = np.where(sel, logits, NEG_INF)
    m = masked.max(1, keepdims=True)
    ex = np.exp(masked - m)
    router_probs = ex / ex.sum(1, keepdims=True)        # [N, E]

    # capacity ranks in (k, n) order
    exp_mask = np.zeros((TOPK, N, E), np.int64)
    kk = np.arange(TOPK)[:, None]
    nn = np.arange(N)[None, :]
    exp_mask[kk, nn, topk_idx.T] = 1
    flat = exp_mask.reshape(TOPK * N, E)
    rank = np.cumsum(flat, axis=0) - 1                  # [K*N, E]
    keep = (flat == 1) & (rank < CAP)
    kpos, epos = np.nonzero(keep)
    token = kpos % N
    slot = rank[kpos, epos]
    wgt = router_probs[token, epos]

    # pack the expert batches to the observed max load; if only a few rows
    # push one expert past 1024 slots (= 2 full PSUM chunks), keep the device
    # batch at 1024 and run the leftover rows on the host in fp32.
    loads = np.bincount(epos, minlength=E)
    max_load = int(loads.max())
    cap_k64 = max(64, -(-max_load // 64) * 64)
    overflow = int(np.maximum(loads - 1024, 0).sum())
    cap_k = 1024 if (cap_k64 > 1024 and overflow <= 192) \
        else min(CAP, cap_k64)
    if ("expert", cap_k) not in _CACHE:
        _CACHE[("expert", cap_k)] = build_expert(cap_k)

    on_dev = slot < cap_k
    idx_e = np.zeros((E, cap_k), np.int64)
    w_e = np.zeros((E, cap_k), np.float32)
    idx_e[epos[on_dev], slot[on_dev]] = token[on_dev]
    w_e[epos[on_dev], slot[on_dev]] = wgt[on_dev]

    # ---------------- launch B ----------------
    xln2_bf = _bf16(xln2)
    exp_fc_w = np.asarray(exp_fc_w, np.float32)
    exp_fc_b = np.asarray(exp_fc_b, np.float32).reshape(E, F)
    exp_proj_w = np.asarray(exp_proj_w, np.float32)
    exp_proj_b = np.asarray(exp_proj_b, np.float32).reshape(E, C)

    in_maps_b = []
    for e in range(E):
        xbT = _kperm(np.ascontiguousarray(xln2_bf[idx_e[e]].T))
        fcw = _bf16(exp_fc_w[e]).reshape(CSUB, P, KSUB_F, P)
        fcw = np.ascontiguousarray(fcw.transpose(2, 1, 0, 3))
        pjw = _bf16(exp_proj_w[e]).reshape(KSUB_F, P, CSUB, P)
        pjw = np.ascontiguousarray(pjw.transpose(2, 1, 0, 3))
        in_maps_b.append({
            "xbT": xbT,
            "fcw": fcw,
            "fcb": _pcol(exp_fc_b[e], KSUB_F),
            "pjw": pjw,
            "pjb": _pcol(exp_proj_b[e], CSUB),
        })

    res_b = bass_utils.run_bass_kernel_spmd(
        _CACHE[("expert", cap_k)], in_maps_b, core_ids=list(range(N_CORES)))

    y = xf2.copy()
    for e in range(E):
        valid = w_e[e] != 0
        y[idx_e[e, valid]] += (w_e[e, valid, None]
                               * res_b.results[e]["outT"].T[valid])

    # host top-up for the few rows beyond cap_k (exact fp32)
    if not on_dev.all():
        try:
            from scipy.special import erf
        except ImportError:
            erf = np.vectorize(math.erf)
        off = ~on_dev
        for e in np.unique(epos[off]):
            m = off & (epos == e)
            tk = token[m]
            h = xln2[tk] @ exp_fc_w[e] + exp_fc_b[e]
            h = 0.5 * h * (1.0 + erf(h / math.sqrt(2.0)))
            o = h @ exp_proj_w[e] + exp_proj_b[e]
            y[tk] += wgt[m, None] * o
    return y.reshape(B, T, C).astype(np.float32)


# revision 31
# speedup vs baseline: 1.2192x; 1.0064x over previous
"""MoE transformer block on 8 Trainium2 cores.

Layer: x = x + attn(ln1(x)); x = x + moe(ln2(x)).
Shapes: B=4, T=1024, C=768, H=12 heads, E=8 experts, top-2, cap=1280, F=3072.

Distribution:
  Launch A (attention): core i -> batch i//2, heads 6*(i%2) .. +6.
    LN1 affine is folded into the QKV weights host-side; each core emits a
    partial (6-head) output projection, transposed [C, T], f32. Host sums the
    two half-head partials per batch and adds the residual.
  Host: ln2 + gating + exact top-2 capacity routing (numpy, matches the jax
    reference in ordering; near-tie tokens get exact fp32 logits), builds
    per-expert gather indices.
  Launch B (experts): core e -> expert e, slots packed to the observed max
    load (rounded up to 64). xbT [C, cap_k] bf16 in, outT [C, cap_k] f32 out.
    Host scatter-adds w * out into y (per-expert indices are unique, so
    fancy-index += is collision-free).
"""

import math

import numpy as np
import ml_dtypes

import concourse.bacc as bacc
import concourse.bass as bass
import concourse.mybir as mybir
import concourse.tile as tile
from concourse import bass_utils
from concourse.masks import make_identity

F32 = mybir.dt.float32
BF16 = mybir.dt.bfloat16
AF = mybir.ActivationFunctionType
ALU = mybir.AluOpType
AX = mybir.AxisListType

B, T, C = 4, 1024, 768
NHEAD = 12
HD = C // NHEAD  # 64
E = 8
TOPK = 2
CAP = 1280
F = 4 * C  # 3072
LN_EPS = 1e-5
NEG_INF = -1e30
P = 128

N_CORES = 8
H6 = NHEAD // 2          # heads per core
D6 = H6 * HD             # 384
CSUB = C // P            # 6
KSUB_F = F // P          # 24
NT = T // P              # 8
QKV9 = 3 * D6 // P       # 9

_CACHE = {}


def _chunks(n, step=512):
    out = []
    s = 0
    while s < n:
        out.append((s, min(step, n - s)))
        s += step
    return out


# --------------------------------------------------------------------------
# Launch A: attention
# --------------------------------------------------------------------------

def build_attn():
    nc = bacc.Bacc("TRN2", target_bir_lowering=False, debug=False)

    xb = nc.dram_tensor("xb", [T, C], F32, kind="ExternalInput")
    # qkv weight slice for this core's 6 heads, ln1-folded, q pre-scaled by
    # 1/sqrt(HD), pre-permuted to [p, ks, n]. column order within n:
    # q h0..h5 | k h0..h5 | v h0..h5 (64 cols each head)
    wqkv = nc.dram_tensor("wqkv", [P, CSUB, 3 * D6], BF16, kind="ExternalInput")
    bqkv = nc.dram_tensor("bqkv", [P, QKV9], F32, kind="ExternalInput")
    wpj = nc.dram_tensor("wpj", [P, D6 // P, C], BF16, kind="ExternalInput")
    bpj = nc.dram_tensor("bpj", [P, CSUB], F32, kind="ExternalInput")
    # transposed causal mask (bf16): cmaskT[k, q] = 0 if k <= q else -1e30
    cmaskT = nc.dram_tensor("cmaskT", [P, P], BF16, kind="ExternalInput")
    out = nc.dram_tensor("attn_pT", [C, T], F32, kind="ExternalOutput")

    with tile.TileContext(nc) as tc:
        with (
            tc.tile_pool(name="const", bufs=1) as const,
            tc.tile_pool(name="xin", bufs=1) as xin,
            tc.tile_pool(name="big", bufs=1) as big,
            tc.tile_pool(name="pTp", bufs=2) as pTp,
            tc.tile_pool(name="work", bufs=4) as work,
            tc.tile_pool(name="ps", bufs=2, space="PSUM") as ps,
            tc.tile_pool(name="ps_t", bufs=1, space="PSUM") as ps_t,
            tc.tile_pool(name="ps_y", bufs=1, space="PSUM") as ps_y,
        ):
            # x tiles first: they gate the whole pipeline, so their DMAs must
            # not queue behind the (larger) weight loads
            xts = []
            for ti in range(NT):
                xt = xin.tile([P, C], F32, tag=f"x{ti}", name=f"x{ti}")
                nc.sync.dma_start(xt[:], xb[ti * P:(ti + 1) * P, :])
                xts.append(xt)

            ident = const.tile([P, P], BF16)
            make_identity(nc, ident[:])
            cm = const.tile([P, P], BF16)
            nc.sync.dma_start(cm[:], cmaskT[:])
            wqkv_sb = const.tile([P, CSUB, 3 * D6], BF16)
            nc.sync.dma_start(wqkv_sb[:], wqkv[:])
            bqkv_sb = const.tile([P, QKV9], F32)
            nc.sync.dma_start(bqkv_sb[:], bqkv[:])
            wpj_sb = const.tile([P, D6 // P, C], BF16)
            nc.sync.dma_start(wpj_sb[:], wpj[:])
            bpj_sb = const.tile([P, CSUB], F32)
            nc.sync.dma_start(bpj_sb[:], bpj[:])

            # ---- LN1 + transpose -> xlnT [C, T] (two T-half tiles so the
            # qkv matmuls can start after the first half)
            xlnT = [big.tile([P, CSUB, T // 2], BF16, tag=f"xlnT{i}",
                             name=f"xlnT{i}") for i in range(2)]
            for ti in range(NT):
                xt = xts[ti]
                s1 = work.tile([P, 1], F32, tag="s1")
                nc.vector.reduce_sum(s1[:], xt[:], axis=AX.X)
                sq = work.tile([P, C], F32, tag="sq")
                s2 = work.tile([P, 1], F32, tag="s2")
                nc.scalar.activation(sq[:], xt[:], AF.Square, accum_out=s2[:])
                negmu = work.tile([P, 1], F32, tag="negmu")
                nc.vector.tensor_scalar_mul(negmu[:], s1[:], -1.0 / C)
                # var = s2/C + eps - mu^2
                var = work.tile([P, 1], F32, tag="var")
                nc.vector.tensor_scalar(
                    var[:], s2[:], 1.0 / C, LN_EPS, op0=ALU.mult, op1=ALU.add)
                mu2 = work.tile([P, 1], F32, tag="mu2")
                nc.vector.tensor_tensor(mu2[:], negmu[:], negmu[:], op=ALU.mult)
                nc.vector.tensor_tensor(var[:], var[:], mu2[:], op=ALU.subtract)
                std = work.tile([P, 1], F32, tag="std")
                nc.scalar.activation(std[:], var[:], AF.Sqrt)
                rstd = work.tile([P, 1], F32, tag="rstd")
                nc.vector.reciprocal(rstd[:], std[:])
                xn = work.tile([P, C], BF16, tag="xn")
                nc.gpsimd.tensor_scalar(
                    xn[:], xt[:], negmu[:], rstd[:],
                    op0=ALU.add, op1=ALU.mult)
                pt = ps_t.tile([P, CSUB, P], BF16, tag="pt6")
                for cs in range(CSUB):
                    nc.tensor.transpose(
                        pt[:, cs, :], xn[:, cs * P:(cs + 1) * P], ident[:])
                nc.vector.tensor_copy(
                    xlnT[ti // 4][:, :, (ti % 4) * P:(ti % 4 + 1) * P], pt[:])

            # ---- qkvT [3*D6, T] = wqkv.T @ xln.T, + bias
            # one SBUF tile per 128-row group so consumers wait only on the
            # rows they read, letting the head loop overlap this phase
            qkvT = [big.tile([P, T], BF16, tag=f"qkvT{mc}", name=f"qkvT{mc}")
                    for mc in range(QKV9)]
            v_ones = big.tile([P, NT, H6, 1 + 64], BF16)
            nc.vector.memset(v_ones[:, :, :, 0:1], 1.0)
            y_big = big.tile([P, NT, D6], BF16)

            def emit_qkv(mc):
                for th in range(T // 512):
                    pacc = ps.tile([P, 512], F32, tag="mm", name=f"qk{mc}{th}")
                    for ks in range(CSUB):
                        nc.tensor.matmul(
                            pacc[:],
                            lhsT=wqkv_sb[:, ks, mc * P:(mc + 1) * P],
                            rhs=xlnT[th][:, ks, :],
                            start=(ks == 0), stop=(ks == CSUB - 1))
                    nc.scalar.activation(
                        qkvT[mc][:, th * 512:(th + 1) * 512], pacc[:],
                        AF.Identity, bias=bqkv_sb[:, mc:mc + 1])

            def emit_vones(j):
                # vT row j -> v for heads 2j, 2j+1 (col 0 stays all-ones)
                for ti in range(NT):
                    pt6 = ps_t.tile([P, CSUB, P], BF16, tag="pt6",
                                    name=f"vt{j}{ti}")
                    nc.tensor.transpose(
                        pt6[:, 0, :],
                        qkvT[2 * (D6 // P) + j][:, ti * P:(ti + 1) * P],
                        ident[:])
                    nc.vector.tensor_copy(
                        v_ones[:, ti, 2 * j:2 * j + 2, 1:],
                        pt6[:, 0, :].rearrange("p (a b) -> p a b", a=2))

            def emit_head(h):
                # scores transposed sT[k, q] so Exp lands pT in SBUF directly;
                # AV fuses the softmax denominator via v_ones col 0; the
                # causal mask of the diagonal block is added by the PE itself.
                qp0 = 64 * (h % 2)
                qrow = h // 2
                kp0 = (D6 + 64 * h) % P
                krow = (D6 + 64 * h) // P
                pT = pTp.tile([P, NT, T], BF16, tag="pT", name=f"pT{h}")
                for kb in range(NT):
                    q0 = kb * P
                    pscore = ps.tile([P, T], F32, tag="sc", name=f"sc{h}{kb}")
                    # chunk on absolute 512 boundaries (PSUM bank alignment)
                    bounds = [q0] + [b for b in (512, T) if b > q0]
                    for (s0, e0) in zip(bounds[:-1], bounds[1:]):
                        w = e0 - s0
                        nc.tensor.matmul(
                            pscore[:, s0:s0 + w],
                            lhsT=qkvT[krow][kp0:kp0 + 64, kb * P:(kb + 1) * P],
                            rhs=qkvT[qrow][qp0:qp0 + 64, s0:s0 + w],
                            start=True, stop=True)
                    nc.tensor.matmul(
                        pscore[:, q0:q0 + P], lhsT=ident[:], rhs=cm[:],
                        start=False, stop=True, skip_group_check=True)
                    nc.scalar.activation(
                        pT[:, kb, q0:], pscore[:, q0:], AF.Exp)
                for qi in range(NT):
                    py = ps_y.tile([P, 65], F32, tag="py", name=f"py{h}{qi}")
                    for kb in range(qi + 1):
                        nc.tensor.matmul(
                            py[:], lhsT=pT[:, kb, qi * P:(qi + 1) * P],
                            rhs=v_ones[:, kb, h, :],
                            start=(kb == 0), stop=(kb == qi))
                    rec = work.tile([P, 1], F32, tag="rec")
                    nc.vector.reciprocal(rec[:], py[:, 0:1])
                    nc.vector.tensor_tensor(
                        y_big[:, qi, h * 64:(h + 1) * 64], py[:, 1:],
                        rec[:].to_broadcast([P, 64]), op=ALU.mult)

            # interleave: emit each head-pair's q/k/v columns, its v
            # transposes, then its two heads, so ACT's exp work starts while
            # the PE is still on later qkv matmuls
            for g in range(D6 // P):
                emit_qkv(g)
                emit_qkv(3 + g)
                emit_qkv(6 + g)
                emit_vones(g)
                emit_head(2 * g)
                emit_head(2 * g + 1)

            # ---- yT [D6, T] in two T-half tiles (proj starts on half 0
            # while the last head still fills half 1)
            yT = [big.tile([P, D6 // P, T // 2], BF16, tag=f"yT{i}",
                           name=f"yT{i}") for i in range(2)]
            for qi in range(NT):
                pt6 = ps_t.tile([P, CSUB, P], BF16, tag="pt6")
                pt = pt6[:, :D6 // P, :]
                for j in range(D6 // P):
                    nc.tensor.transpose(
                        pt[:, j, :], y_big[:, qi, j * P:(j + 1) * P], ident[:])
                nc.vector.tensor_copy(
                    yT[qi // 4][:, :, (qi % 4) * P:(qi % 4 + 1) * P], pt[:])

            # ---- partial projection: outT [C, T] = wpj.T @ y.T + bpj
            for cc in range(CSUB):
                o_sb = work.tile([P, T], F32, tag="osb")
                for th in range(T // 512):
                    pacc = ps.tile([P, 512], F32, tag="mm", name=f"pj{cc}{th}")
                    for j in range(D6 // P):
                        nc.tensor.matmul(
                            pacc[:],
                            lhsT=wpj_sb[:, j, cc * P:(cc + 1) * P],
                            rhs=yT[th][:, j, :],
                            start=(j == 0), stop=(j == D6 // P - 1))
                    nc.scalar.activation(
                        o_sb[:, th * 512:(th + 1) * 512], pacc[:],
                        AF.Identity, bias=bpj_sb[:, cc:cc + 1])
                nc.sync.dma_start(out[cc * P:(cc + 1) * P, :], o_sb[:])

    nc.compile()
    return nc


# --------------------------------------------------------------------------
# Launch B: experts
# --------------------------------------------------------------------------

def build_expert(cap_k):
    nc = bacc.Bacc("TRN2", target_bir_lowering=False, debug=False)

    xbT = nc.dram_tensor("xbT", [P, CSUB, cap_k], BF16, kind="ExternalInput")
    fcw = nc.dram_tensor("fcw", [KSUB_F, P, CSUB, P], BF16,
                         kind="ExternalInput")
    fcb = nc.dram_tensor("fcb", [P, KSUB_F], F32, kind="ExternalInput")
    pjw = nc.dram_tensor("pjw", [CSUB, P, KSUB_F, P], BF16,
                         kind="ExternalInput")
    pjb = nc.dram_tensor("pjb", [P, CSUB], F32, kind="ExternalInput")
    out = nc.dram_tensor("outT", [C, cap_k], F32, kind="ExternalOutput")

    SC = _chunks(cap_k)

    with tile.TileContext(nc) as tc:
        with (
            tc.tile_pool(name="const", bufs=1) as const,
            tc.tile_pool(name="w1", bufs=6) as w1p,
            tc.tile_pool(name="w2", bufs=4) as w2p,
            tc.tile_pool(name="big", bufs=1) as big,
            tc.tile_pool(name="osb", bufs=2) as osbp,
            tc.tile_pool(name="ps", bufs=4, space="PSUM") as ps,
        ):
            xbT_sb = const.tile([P, CSUB, cap_k], BF16)
            for (s0, sw) in SC:
                nc.sync.dma_start(
                    xbT_sb[:, :, s0:s0 + sw], xbT[:, :, s0:s0 + sw])
            fcb_sb = const.tile([P, KSUB_F], F32)
            nc.sync.dma_start(fcb_sb[:], fcb[:])
            pjb_sb = const.tile([P, CSUB], F32)
            nc.sync.dma_start(pjb_sb[:], pjb[:])

            hT = big.tile([P, KSUB_F, cap_k], BF16)
            for mf in range(KSUB_F):
                wt = w1p.tile([P, CSUB, P], BF16, tag="w1")
                nc.sync.dma_start(wt[:], fcw[mf])
                for (s0, sw) in SC:
                    pacc = ps.tile([P, 512], F32, tag="mm")
                    for ks in range(CSUB):
                        nc.tensor.matmul(
                            pacc[:, :sw], lhsT=wt[:, ks, :],
                            rhs=xbT_sb[:, ks, s0:s0 + sw],
                            start=(ks == 0), stop=(ks == CSUB - 1))
                    nc.scalar.activation(
                        hT[:, mf, s0:s0 + sw], pacc[:, :sw],
                        AF.Gelu, bias=fcb_sb[:, mf:mf + 1])

            for cc in range(CSUB):
                wt = w2p.tile([P, KSUB_F, P], BF16, tag="w2")
                nc.sync.dma_start(wt[:], pjw[cc])
                o_sb = osbp.tile([P, cap_k], F32, tag="osb")
                for (s0, sw) in SC:
                    pacc = ps.tile([P, 512], F32, tag="mm")
                    for ks in range(KSUB_F):
                        nc.tensor.matmul(
                            pacc[:, :sw], lhsT=wt[:, ks, :],
                            rhs=hT[:, ks, s0:s0 + sw],
                            start=(ks == 0), stop=(ks == KSUB_F - 1))
                    nc.scalar.activation(
                        o_sb[:, s0:s0 + sw], pacc[:, :sw],
                        AF.Identity, bias=pjb_sb[:, cc:cc + 1])
                nc.sync.dma_start(out[cc * P:(cc + 1) * P, :], o_sb[:])

    nc.compile()
    return nc


# --------------------------------------------------------------------------
# Host glue
# --------------------------------------------------------------------------

def _bf16(a):
    return np.asarray(a, np.float32).astype(ml_dtypes.bfloat16)


def _pcol(vec, nsub):
    """[nsub*P] -> [P, nsub] per-partition bias layout."""
    return np.ascontiguousarray(
        np.asarray(vec, np.float32).reshape(nsub, P).T)


def _kperm(w):
    """[K, N] -> [P, K//P, N] partition-major layout, contiguous."""
    k, n = w.shape
    return np.ascontiguousarray(w.reshape(k // P, P, n).transpose(1, 0, 2))


def _layer_norm(x, w, b):
    mu = x.mean(-1, keepdims=True)
    var = x.var(-1, keepdims=True)
    return (x - mu) / np.sqrt(var + LN_EPS) * w + b


def _exact_logits(need, x, ln1_w, ln1_b, ln2_w, ln2_b, qkv_w, qkv_b,
                  proj_w, proj_b, w_g):
    """fp32 gating logits for the given flat token indices (exact attention
    rows for just those tokens)."""
    out = np.empty((need.size, E), np.float32)
    bs, ps = need // T, need % T
    for b in np.unique(bs):
        m = bs == b
        pos = ps[m]                              # [M]
        xl = _layer_norm(x[b], ln1_w, ln1_b)     # [T, C]
        kv = xl @ qkv_w[:, C:] + qkv_b[C:]       # [T, 2C]
        k = kv[:, :C].reshape(T, NHEAD, HD)
        v = kv[:, C:].reshape(T, NHEAD, HD)
        q = (xl[pos] @ qkv_w[:, :C] + qkv_b[:C]).reshape(-1, NHEAD, HD)
        s = np.einsum("mhd,khd->mhk", q, k) / math.sqrt(HD)
        s = np.where(pos[:, None, None] >= np.arange(T)[None, None, :],
                     s, NEG_INF)
        s -= s.max(-1, keepdims=True)
        p = np.exp(s)
        p /= p.sum(-1, keepdims=True)
        y = np.einsum("mhk,khd->mhd", p, v).reshape(-1, C)
        att = y @ proj_w + proj_b
        x2 = x[b][pos] + att
        out[m] = _layer_norm(x2, ln2_w, ln2_b) @ w_g
    return out


def kernel(x, ln1_w, ln1_b, ln2_w, ln2_b, attn_qkv_w, attn_qkv_b,
           attn_proj_w, attn_proj_b, w_g, exp_fc_w, exp_fc_b,
           exp_proj_w, exp_proj_b):
    x = np.asarray(x, np.float32)
    ln1_w = np.asarray(ln1_w, np.float32)
    ln1_b = np.asarray(ln1_b, np.float32)
    attn_qkv_w = np.asarray(attn_qkv_w, np.float32)
    attn_qkv_b = np.asarray(attn_qkv_b, np.float32)
    attn_proj_w = np.asarray(attn_proj_w, np.float32)
    attn_proj_b = np.asarray(attn_proj_b, np.float32)

    if "attn" not in _CACHE:
        _CACHE["attn"] = build_attn()

    # ---------------- launch A ----------------
    # fold ln1 affine into qkv: qkv = xhat @ (diag(w1) W) + (b1 @ W + b)
    Wf = ln1_w[:, None] * attn_qkv_w          # [C, 3C]
    bf = ln1_b @ attn_qkv_w + attn_qkv_b      # [3C]
    Wq = Wf[:, :C] / math.sqrt(HD)
    bq = bf[:C] / math.sqrt(HD)
    Wk, bk = Wf[:, C:2 * C], bf[C:2 * C]
    Wv, bv = Wf[:, 2 * C:], bf[2 * C:]

    cmaskT_np = _bf16(np.where(
        np.triu(np.ones((P, P), bool)), 0.0, NEG_INF))

    in_maps_a = []
    for core in range(N_CORES):
        b = core // 2
        h0 = H6 * (core % 2)
        cols = slice(h0 * HD, (h0 + H6) * HD)
        wqkv_c = np.concatenate([Wq[:, cols], Wk[:, cols], Wv[:, cols]], 1)
        bqkv_c = np.concatenate([bq[cols], bk[cols], bv[cols]])
        bpj_c = attn_proj_b if core % 2 == 0 else np.zeros(C, np.float32)
        in_maps_a.append({
            "xb": np.ascontiguousarray(x[b]),
            "wqkv": _kperm(_bf16(wqkv_c)),
            "bqkv": _pcol(bqkv_c, QKV9),
            "wpj": _kperm(_bf16(attn_proj_w[h0 * HD:(h0 + H6) * HD, :])),
            "bpj": _pcol(bpj_c, CSUB),
            "cmaskT": cmaskT_np,
        })

    res_a = bass_utils.run_bass_kernel_spmd(
        _CACHE["attn"], in_maps_a, core_ids=list(range(N_CORES)))

    attn = np.empty((B, T, C), np.float32)
    for b in range(B):
        attn[b] = (res_a.results[2 * b]["attn_pT"]
                   + res_a.results[2 * b + 1]["attn_pT"]).T

    x2 = x + attn                       # [B, T, C]
    xf2 = x2.reshape(B * T, C)

    # ---------------- host routing (exact reference semantics) -------------
    N = B * T
    xln2 = _layer_norm(xf2, np.asarray(ln2_w, np.float32),
                       np.asarray(ln2_b, np.float32))
    logits = xln2 @ np.asarray(w_g, np.float32)        # [N, E]

    # The top-2 expert choice is discontinuous: tokens whose top2/top3 gating
    # logits are within the bf16 noise floor could route differently than the
    # fp32 reference would. Recompute those few tokens' logits exactly.
    srt = np.sort(logits, axis=1)
    need = np.nonzero(srt[:, -2] - srt[:, -3] < 0.02)[0]
    if need.size:
        logits[need] = _exact_logits(
            need, x, ln1_w, ln1_b, np.asarray(ln2_w, np.float32),
            np.asarray(ln2_b, np.float32), attn_qkv_w, attn_qkv_b,
            attn_proj_w, attn_proj_b, np.asarray(w_g, np.float32))

    order = np.argsort(-logits, axis=1, kind="stable")
    topk_idx = order[:, :TOPK]                          # [N, K]
    sel = np.zeros((N, E), bool)
    np.put_along_axis(sel, topk_idx, True, axis=1)
    masked = np.where(sel, logits, NEG_INF)
    m = masked.max(1, keepdims=True)
    ex = np.exp(masked - m)
    router_probs = ex / ex.sum(1, keepdims=True)        # [N, E]

    # capacity ranks in (k, n) order
    exp_mask = np.zeros((TOPK, N, E), np.int64)
    kk = np.arange(TOPK)[:, None]
    nn = np.arange(N)[None, :]
    exp_mask[kk, nn, topk_idx.T] = 1
    flat = exp_mask.reshape(TOPK * N, E)
    rank = np.cumsum(flat, axis=0) - 1                  # [K*N, E]
    keep = (flat == 1) & (rank < CAP)
    kpos, epos = np.nonzero(keep)
    token = kpos % N
    slot = rank[kpos, epos]
    wgt = router_probs[token, epos]

    # pack the expert batches to the observed max load; if only a few rows
    # push one expert past 1024 slots (= 2 full PSUM chunks), keep the device
    # batch at 1024 and run the leftover rows on the host in fp32.
    loads = np.bincount(epos, minlength=E)
    max_load = int(loads.max())
    cap_k64 = max(64, -(-max_load // 64) * 64)
    overflow = int(np.maximum(loads - 1024, 0).sum())
    cap_k = 1024 if (cap_k64 > 1024 and overflow <= 192) \
        else min(CAP, cap_k64)
    if ("expert", cap_k) not in _CACHE:
        _CACHE[("expert", cap_k)] = build_expert(cap_k)

    on_dev = slot < cap_k
    idx_e = np.zeros((E, cap_k), np.int64)
    w_e = np.zeros((E, cap_k), np.float32)
    idx_e[epos[on_dev], slot[on_dev]] = token[on_dev]
    w_e[epos[on_dev], slot[on_dev]] = wgt[on_dev]

    # ---------------- launch B ----------------
    xln2_bf = _bf16(xln2)
    exp_fc_w = np.asarray(exp_fc_w, np.float32)
    exp_fc_b = np.asarray(exp_fc_b, np.float32).reshape(E, F)
    exp_proj_w = np.asarray(exp_proj_w, np.float32)
    exp_proj_b = np.asarray(exp_proj_b, np.float32).reshape(E, C)

    in_maps_b = []
    for e in range(E):
        xbT = _kperm(np.ascontiguousarray(xln2_bf[idx_e[e]].T))
        fcw = _bf16(exp_fc_w[e]).reshape(CSUB, P, KSUB_F, P)
        fcw = np.ascontiguousarray(fcw.transpose(2, 1, 0, 3))
        pjw = _bf16(exp_proj_w[e]).reshape(KSUB_F, P, CSUB, P)
        pjw = np.ascontiguousarray(pjw.transpose(2, 1, 0, 3))
        in_maps_b.append({
            "xbT": xbT,
            "fcw": fcw,
            "fcb": _pcol(exp_fc_b[e], KSUB_F),
            "pjw": pjw,
            "pjb": _pcol(exp_proj_b[e], CSUB),
        })

    res_b = bass_utils.run_bass_kernel_spmd(
        _CACHE[("expert", cap_k)], in_maps_b, core_ids=list(range(N_CORES)))

    y = xf2.copy()
    for e in range(E):
        valid = w_e[e] != 0
        y[idx_e[e, valid]] += (w_e[e, valid, None]
                               * res_b.results[e]["outT"].T[valid])

    # host top-up for the few rows beyond cap_k (exact fp32)
    if not on_dev.all():
        try:
            from scipy.special import erf
        except ImportError:
            erf = np.vectorize(math.erf)
        off = ~on_dev
        for e in np.unique(epos[off]):
            m = off & (epos == e)
            tk = token[m]
            h = xln2[tk] @ exp_fc_w[e] + exp_fc_b[e]
            h = 0.5 * h * (1.0 + erf(h / math.sqrt(2.0)))
            o = h @ exp_proj_w[e] + exp_proj_b[e]
            y[tk] += wgt[m, None] * o
    return y.reshape(B, T, C).astype(np.float32)
